# revision 17
# baseline (speedup 1.0000x reference)
"""DeepGraphSAGE (4x SAGEConv + BN/ReLU) on 8 Trainium2 NeuronCores.

Sharding: nodes partitioned across 8 cores (6250 dst nodes each). Per layer:
  - mean-aggregate neighbor features via dma_gather + one-hot selection
    matmuls accumulating in PSUM (S built on-chip via iota/is_equal).
  - edges 3-way split by source: SELF (own rows, gathered from local h_own
    during the collective window, pre-aggregated into DRAM partials),
    REMOTE-A (first HALF local rows of each owner) and REMOTE-B (rest).
    The hidden-state exchange is two sub-AllGathers (A then B) so remote-A
    gathers start as soon as the first half arrives; all index spaces fit
    int16 without a base split.
  - dense transforms in transposed layout (features on partitions)
  - BatchNorm stats via accumulated sums + tiny cross-core AllReduce
  - next layer's root term precomputed on PE during the collectives.
Hidden tables travel fp8e3 (E3M4); weights/root f16; accumulation fp32.
"""
import sys
import numpy as np

for p in ("/opt/trn_rl_repo",):
    if p not in sys.path:
        sys.path.append(p)

import concourse.bass as bass
import concourse.bacc as bacc
import concourse.mybir as mybir
from concourse.tile import TileContext
from concourse.masks import make_identity

f32 = mybir.dt.float32
f16 = mybir.dt.float16
fp8 = mybir.dt.float8e3
i16 = mybir.dt.int16

NCORES = 8
P = 128
HALF_BLK = 24
HALF = HALF_BLK * P            # 3072 local rows in the A half
EPS = 1e-5
LAST_BUILD = None
TAB_DT = {1: fp8, 2: fp8}


# ---------------------------------------------------------------- host prep
class Plan:
    """Per-core gather plan: self / remote-A / remote-B chunk streams."""

    def __init__(self, n_nodes, src, dst, core):
        nown = n_nodes // NCORES
        self.n_own = nown
        self.nblk = (nown + P - 1) // P
        self.npair = (self.nblk + 1) // 2
        nhB = nown - HALF
        lo = core * nown
        m = (dst >= lo) & (dst < lo + nown)
        es = src[m].astype(np.int64)
        ed = (dst[m] - lo).astype(np.int64)
        order = np.argsort(ed, kind="stable")
        es, ed = es[order], ed[order]
        bounds = np.searchsorted(ed, np.arange(0, self.nblk + 1) * P)
        owner = es // nown
        lr = es % nown
        grp = np.where(owner == core, 0, np.where(lr < HALF, 1, 2))
        gidx = np.where(grp == 0, lr,
                        np.where(grp == 1, owner * HALF + lr,
                                 owner * nhB + (lr - HALF)))
        # sitems[pr][bi] / ritems[pr][gi][bi] = (idx_flat[k*P], dst[P,k])
        self.sitems, self.ritems = [], []
        for pr in range(self.npair):
            blocks = [b for b in (2 * pr, 2 * pr + 1) if b < self.nblk]
            self.sitems.append(
                [self._chunk(es, ed, bounds, b, grp, gidx, 0) for b in blocks])
            self.ritems.append(
                [[self._chunk(es, ed, bounds, b, grp, gidx, gid)
                  for b in blocks] for gid in (1, 2)])

    @staticmethod
    def _chunk(es, ed, bounds, b, grp, gidx, gid):
        e0, e1 = bounds[b], bounds[b + 1]
        msel = grp[e0:e1] == gid
        gs = gidx[e0:e1][msel]
        gd = ed[e0:e1][msel] - b * P
        k = (len(gs) + P - 1) // P
        iv = np.zeros(k * P, np.int16)
        iv[:len(gs)] = gs.astype(np.int16)
        dv = np.full((P, k), -1, np.int16)
        if k:
            dpad = np.full(k * P, -1, np.int64)
            dpad[:len(gd)] = gd
            dv = dpad.reshape(k, P).T.astype(np.int16).copy()
        return iv, dv


def _pack(idx_vals, dst_vals):
    iv = np.concatenate(idx_vals) if idx_vals else np.zeros(0, np.int16)
    w = iv.reshape(-1, 16).T if iv.size else np.zeros((16, 0), np.int16)
    idx16 = np.tile(w, (8, 1)).copy()
    dst16 = (np.concatenate(dst_vals, axis=1).copy()
             if dst_vals else np.zeros((P, 1), np.int16))
    return idx16, dst16


def _finalize(plans):
    """Cross-core pad chunk counts; build packed idx/dst tables + call lists."""
    npair = plans[0].npair

    def padded(item, km):
        iv, dv = item
        k = dv.shape[1] if dv.size else 0
        iv2 = np.zeros(km * P, np.int16)
        iv2[:k * P] = iv
        dv2 = np.full((P, km), -1, np.int16)
        if k:
            dv2[:, :k] = dv
        return iv2, dv2

    kmax_s = [[max(len(p.sitems[pr][bi][0]) // P for p in plans)
               for bi in range(len(plans[0].sitems[pr]))]
              for pr in range(npair)]
    kmax_r = [[[max(len(p.ritems[pr][gi][bi][0]) // P for p in plans)
                for bi in range(len(plans[0].ritems[pr][gi]))]
               for gi in range(2)] for pr in range(npair)]
    for p in plans:
        sidx, sdst, ridx, rdst = [], [], [], []
        for pr in range(npair):
            for bi, km in enumerate(kmax_s[pr]):
                iv, dv = padded(p.sitems[pr][bi], km)
                if km:
                    sidx.append(iv)
                    sdst.append(dv)
            for gi in range(2):
                for bi, km in enumerate(kmax_r[pr][gi]):
                    iv, dv = padded(p.ritems[pr][gi][bi], km)
                    if km:
                        ridx.append(iv)
                        rdst.append(dv)
        p.idxS, p.dstS = _pack(sidx, sdst)
        p.idxR, p.dstR = _pack(ridx, rdst)
    pl = plans[0]
    pl.scalls = [list(kmax_s[pr]) for pr in range(npair)]
    pl.rcalls = [[(gi + 1, list(kmax_r[pr][gi])) for gi in range(2)]
                 for pr in range(npair)]
    pl.schoff, pl.rchoff = [], []
    so = ro = 0
    for pr in range(npair):
        pl.schoff.append(so)
        pl.rchoff.append(ro)
        so += sum(kmax_s[pr])
        ro += sum(sum(ks) for _, ks in pl.rcalls[pr])
    pl.stot, pl.rtot = so, ro
    return plans


# ---------------------------------------------------------------- program
def build_program(n_nodes, in_f, hid, out_f, pl):
    nown = pl.n_own
    nblk = pl.nblk
    pad_n = nblk * P
    ntile = (nown + 511) // 512
    nfc = hid // P
    npair = pl.npair
    nhB = nown - HALF
    dims = [(in_f, hid), (hid, hid), (hid, hid), (hid, out_f)]

    nc = bacc.Bacc("TRN2", target_bir_lowering=False, debug=False,
                   num_devices=NCORES, num_swdge_queues=4,
                   dynamic_dma_scratch_size=12288)

    # ---- I/O ----
    xT = nc.dram_tensor("xT", [in_f, pad_n], f16, kind="ExternalInput")
    xself_d = nc.dram_tensor("xself", [nown, 128], f16, kind="ExternalInput")
    xA_d = nc.dram_tensor("xA", [NCORES * HALF, 128], f16, kind="ExternalInput")
    xB_d = nc.dram_tensor("xB", [NCORES * nhB, 128], f16, kind="ExternalInput")
    idxS_d = nc.dram_tensor("idxS", [P, max(pl.stot * 8, 8)], i16, kind="ExternalInput")
    dstS_d = nc.dram_tensor("dstS", [P, max(pl.stot, 1)], i16, kind="ExternalInput")
    idxR_d = nc.dram_tensor("idxR", [P, max(pl.rtot * 8, 8)], i16, kind="ExternalInput")
    dstR_d = nc.dram_tensor("dstR", [P, max(pl.rtot, 1)], i16, kind="ExternalInput")
    deginv_d = nc.dram_tensor("deginv", [pad_n], f32, kind="ExternalInput")
    wl_d, wr_d, g_d, b_d = {}, {}, {}, {}
    for l, (fi, fo) in enumerate(dims, start=1):
        wl_d[l] = nc.dram_tensor(f"Wl{l}", [fi, fo], f16, kind="ExternalInput")
        wr_d[l] = nc.dram_tensor(f"Wr{l}", [fi, fo], f16, kind="ExternalInput")
    for l in (1, 2, 3):
        g_d[l] = nc.dram_tensor(f"g{l}", [hid], f32, kind="ExternalInput")
        b_d[l] = nc.dram_tensor(f"b{l}", [hid], f32, kind="ExternalInput")
    bl4_d = nc.dram_tensor("bl4", [out_f], f32, kind="ExternalInput")
    out_d = nc.dram_tensor("out", [nown, out_f], f32, kind="ExternalOutput")

    # ---- internal DRAM ----
    h_own = {l: nc.dram_tensor(f"h{l}_own", [nown, hid], TAB_DT[l]) for l in (1, 2)}
    h_gA = {l: nc.dram_tensor(f"h{l}_gA", [NCORES * HALF, hid], TAB_DT[l],
                              addr_space="Shared") for l in (1, 2)}
    h_gB = {l: nc.dram_tensor(f"h{l}_gB", [NCORES * nhB, hid], TAB_DT[l],
                              addr_space="Shared") for l in (1, 2)}
    y_own = nc.dram_tensor("y_own", [nown, 128], f16)
    y_gA = nc.dram_tensor("y_gA", [NCORES * HALF, 128], f16, addr_space="Shared")
    y_gB = nc.dram_tensor("y_gB", [NCORES * nhB, 128], f16, addr_space="Shared")
    spart = {2: nc.dram_tensor("sp2", [pad_n, hid], f16),
             3: nc.dram_tensor("sp3", [pad_n, hid], f16),
             4: nc.dram_tensor("sp4", [pad_n, 128], f16)}
    st_in = {l: nc.dram_tensor(f"st{l}_in", [P, 8], f32) for l in (1, 2, 3)}
    st_out = {l: nc.dram_tensor(f"st{l}_out", [P, 8], f32, addr_space="Shared")
              for l in (1, 2, 3)}
    rg = [list(range(NCORES))]

    with TileContext(nc) as tc:
        with (
            tc.tile_pool(name="const", bufs=1) as cp,
            tc.tile_pool(name="sbuf", bufs=2) as sb,
            tc.tile_pool(name="small", bufs=3) as sm,
            tc.tile_pool(name="spool", bufs=3) as sp,
            tc.tile_pool(name="gpool", bufs=3) as gp,
            tc.tile_pool(name="sspool", bufs=2) as ssp,
            tc.tile_pool(name="sgpool", bufs=2) as sgp,
            tc.tile_pool(name="ppool", bufs=2) as pp,
            tc.tile_pool(name="rows", bufs=2) as rp,
            tc.tile_pool(name="psA", bufs=2, space="PSUM") as psA,
            tc.tile_pool(name="psB", bufs=2, space="PSUM") as psB,
            tc.tile_pool(name="psC", bufs=2, space="PSUM") as psC,
        ):
            ident = cp.tile([P, P], f16)
            make_identity(nc, ident[:])
            ident32 = cp.tile([P, P], f32)
            make_identity(nc, ident32[:])
            iota_t = cp.tile([P, P], i16)
            nc.gpsimd.iota(iota_t[:], pattern=[[1, P]], base=0,
                           channel_multiplier=0,
                           allow_small_or_imprecise_dtypes=True)
            deginv_t = cp.tile([P, nblk], f32)
            nc.sync.dma_start(out=deginv_t[:],
                              in_=deginv_d[:].rearrange("(b p) -> p b", p=P))
            idxSc = cp.tile([P, max(pl.stot * 8, 8)], i16)
            nc.sync.dma_start(out=idxSc[:], in_=idxS_d[:, :])
            dstSc = cp.tile([P, max(pl.stot, 1)], i16)
            nc.sync.dma_start(out=dstSc[:], in_=dstS_d[:, :])
            idxRc = cp.tile([P, max(pl.rtot * 8, 8)], i16)
            nc.sync.dma_start(out=idxRc[:], in_=idxR_d[:, :])
            dstRc = cp.tile([P, max(pl.rtot, 1)], i16)
            nc.sync.dma_start(out=dstRc[:], in_=dstR_d[:, :])
            W = {}
            for l, (fi, fo) in enumerate(dims, start=1):
                kc = (fi + P - 1) // P
                for (nm, dram) in (("l", wl_d[l]), ("r", wr_d[l])):
                    for q in range(kc):
                        r0, r1 = q * P, min((q + 1) * P, fi)
                        t = cp.tile([r1 - r0, fo], f16, tag=f"W{nm}{l}_{q}")
                        nc.sync.dma_start(out=t[:], in_=dram[r0:r1, :])
                        W[(nm, l, q)] = t
            gb = {}
            for l in (1, 2, 3):
                for nm, dram in (("g", g_d[l]), ("b", b_d[l])):
                    t = cp.tile([P, nfc], f32, tag=f"{nm}{l}")
                    nc.sync.dma_start(out=t[:], in_=dram[:].rearrange("(c p) -> p c", p=P))
                    gb[(nm, l)] = t
            bl4_t = cp.tile([P, 1], f32)
            nc.sync.dma_start(out=bl4_t[:out_f, :], in_=bl4_d[:, None])

            hT = [cp.tile([P, pad_n], f16, tag=f"hT{q}", name=f"hT{q}") for q in range(nfc)]
            preBN = [cp.tile([P, pad_n], f16, tag=f"preBN{q}", name=f"preBN{q}") for q in range(nfc)]
            if pad_n > nown:
                for q in range(nfc):
                    nc.vector.memset(hT[q][:, nown:pad_n], 0.0)

            qload = [0, 0, 0, 0]  # per-queue gathered-row balance

            def next_queue(rows):
                q = min(range(4), key=lambda i: qload[i])
                qload[q] += rows
                return q

            # ---------------- self partial pipeline ----------------
            def self_pipeline(l, table_ap, width, row_elems, dt):
                """Aggregate SELF chunks from the local table into DRAM
                partials [pad_n, width]; runs inside the collective window."""
                for pr in range(npair):
                    ks = pl.scalls[pr]
                    ktot = sum(ks)
                    ch0 = pl.schoff[pr]
                    blocks = [b for b in (2 * pr, 2 * pr + 1) if b < nblk]
                    sS = gS = None
                    if ktot:
                        sS = ssp.tile([P, ktot, P], dt, tag="Ss")
                        nc.vector.tensor_tensor(
                            out=sS[:],
                            in0=dstSc[:, ch0:ch0 + ktot].unsqueeze(2)
                                .broadcast_to([P, ktot, P]),
                            in1=iota_t[:].unsqueeze(1).broadcast_to([P, ktot, P]),
                            op=mybir.AluOpType.is_equal)
                        gS = sgp.tile([P, ktot, row_elems], dt, tag="Gs")
                        nc.gpsimd.dma_gather(
                            out_ap=gS[:], in_ap=table_ap,
                            idxs_ap=idxSc[:, ch0 * 8:(ch0 + ktot) * 8],
                            num_idxs=ktot * P, num_idxs_reg=ktot * P,
                            elem_size=row_elems, single_packet=False,
                            queue_num=next_queue(ktot * P))
                    koff = 0
                    for bi, b in enumerate(blocks):
                        k = ks[bi]
                        zsb = pp.tile([P, width], f16, tag="pout",
                                      name=f"pout{bi}")
                        if k == 0:
                            nc.vector.memset(zsb[:], 0.0)
                        else:
                            ps = psA.tile([P, 512], f32, tag=f"agg_ps{bi}",
                                          name=f"psS{bi}")
                            for n_, j in enumerate(range(koff, koff + k)):
                                nc.tensor.matmul(
                                    out=ps[:, :width], lhsT=sS[:, j, :],
                                    rhs=gS[:, j, :width],
                                    start=(n_ == 0), stop=(n_ == k - 1))
                            if b % 2 == 0:
                                nc.scalar.activation(
                                    out=zsb[:], in_=ps[:, :width],
                                    func=mybir.ActivationFunctionType.Copy)
                            else:
                                nc.vector.tensor_copy(out=zsb[:],
                                                      in_=ps[:, :width])
                        nc.sync.dma_start(
                            out=spart[l][b * P:(b + 1) * P, :width],
                            in_=zsb[:])
                        koff += k

            # ---------------- aggregation ----------------
            def aggregate_pair(pr, tabA, tabB, width, tagsfx, row_elems, dt,
                               selftab=None, seed_l=None):
                """Mean-aggregate both blocks of pair pr from remote tables,
                plus either inline self chunks (selftab) or a DRAM partial
                seed (seed_l). Returns f16 tiles per block."""
                groups = pl.rcalls[pr]
                blocks = [b for b in (2 * pr, 2 * pr + 1) if b < nblk]
                ch0 = pl.rchoff[pr]
                rktot = sum(sum(ks) for _, ks in groups)
                out_tiles = []
                stile = g = None
                if rktot:
                    stile = sp.tile([P, rktot, P], dt, tag="S")
                    nc.vector.tensor_tensor(
                        out=stile[:],
                        in0=dstRc[:, ch0:ch0 + rktot].unsqueeze(2)
                            .broadcast_to([P, rktot, P]),
                        in1=iota_t[:].unsqueeze(1).broadcast_to([P, rktot, P]),
                        op=mybir.AluOpType.is_equal)
                    g = gp.tile([P, rktot, row_elems], dt, tag="G")
                    koff = 0
                    for gid, ks in groups:
                        k = sum(ks)
                        if k == 0:
                            continue
                        src_ap = tabA if gid == 1 else tabB
                        parts = ([k] if k <= 4 else [(k + 1) // 2, k // 2])
                        po = 0
                        for kp in parts:
                            if kp == 0:
                                continue
                            o = koff + po
                            nc.gpsimd.dma_gather(
                                out_ap=g[:, o:o + kp, :], in_ap=src_ap,
                                idxs_ap=idxRc[:, (ch0 + o) * 8:(ch0 + o + kp) * 8],
                                num_idxs=kp * P, num_idxs_reg=kp * P,
                                elem_size=row_elems, single_packet=False,
                                queue_num=next_queue(kp * P))
                            po += kp
                        koff += k
                # inline self chunks (layer 1)
                sS = gS = None
                sks = pl.scalls[pr]
                sch0 = pl.schoff[pr]
                if selftab is not None and sum(sks):
                    sktot = sum(sks)
                    sS = ssp.tile([P, sktot, P], dt, tag="Ss")
                    nc.vector.tensor_tensor(
                        out=sS[:],
                        in0=dstSc[:, sch0:sch0 + sktot].unsqueeze(2)
                            .broadcast_to([P, sktot, P]),
                        in1=iota_t[:].unsqueeze(1).broadcast_to([P, sktot, P]),
                        op=mybir.AluOpType.is_equal)
                    gS = sgp.tile([P, sktot, row_elems], dt, tag="Gs")
                    nc.gpsimd.dma_gather(
                        out_ap=gS[:], in_ap=selftab,
                        idxs_ap=idxSc[:, sch0 * 8:(sch0 + sktot) * 8],
                        num_idxs=sktot * P, num_idxs_reg=sktot * P,
                        elem_size=row_elems, single_packet=False,
                        queue_num=next_queue(sktot * P))
                for bi, b in enumerate(blocks):
                    # remote chunk js of this block
                    mm_idx = []
                    koff = 0
                    for gid, ks in groups:
                        pre = 0
                        for i2, k2 in enumerate(ks):
                            if i2 == bi:
                                mm_idx += list(range(koff + pre, koff + pre + k2))
                            pre += k2
                        koff += sum(ks)
                    smm = []
                    if sS is not None:
                        pre = 0
                        for i2, k2 in enumerate(sks):
                            if i2 == bi:
                                smm = list(range(pre, pre + k2))
                            pre += k2
                    nmm = len(mm_idx) + len(smm) + (1 if seed_l else 0)
                    if nmm == 0:
                        z = sm.tile([P, width], f16, tag=f"agg{tagsfx}{bi}",
                                    name=f"aggz{bi}")
                        nc.vector.memset(z[:], 0.0)
                        out_tiles.append(z)
                        continue
                    agg_ps = psA.tile([P, 512], f32, tag=f"agg_ps{bi}",
                                      name=f"agg_ps{bi}")
                    mm = 0
                    if seed_l:
                        seed = sm.tile([P, width], f16, tag="seed",
                                       name=f"seed{bi}")
                        nc.sync.dma_start(out=seed[:],
                                          in_=spart[seed_l][b * P:(b + 1) * P, :width])
                        nc.tensor.matmul(out=agg_ps[:, :width], lhsT=ident[:],
                                         rhs=seed[:], start=True,
                                         stop=(nmm == 1))
                        mm = 1
                    for j in smm:
                        nc.tensor.matmul(out=agg_ps[:, :width],
                                         lhsT=sS[:, j, :], rhs=gS[:, j, :width],
                                         start=(mm == 0), stop=(mm == nmm - 1))
                        mm += 1
                    for j in mm_idx:
                        nc.tensor.matmul(out=agg_ps[:, :width],
                                         lhsT=stile[:, j, :], rhs=g[:, j, :width],
                                         start=(mm == 0), stop=(mm == nmm - 1))
                        mm += 1
                    asb = sm.tile([P, width], f16, tag=f"agg{tagsfx}{bi}",
                                  name=f"asb{bi}")
                    if b % 2 == 0:
                        nc.scalar.activation(
                            out=asb[:], in_=agg_ps[:, :width],
                            func=mybir.ActivationFunctionType.Copy,
                            scale=deginv_t[:, b:b + 1])
                    else:
                        nc.vector.tensor_scalar(
                            out=asb[:], in0=agg_ps[:, :width],
                            scalar1=deginv_t[:, b:b + 1], scalar2=None,
                            op0=mybir.AluOpType.mult)
                    out_tiles.append(asb)
                return out_tiles

            # ---------------- one SAGE layer with BN+ReLU ----------------
            def layer_123(l, tabA, tabB, fi_chunks, rhs_for_fi, width,
                          row_elems, dt, root_pre=False, selftab=None,
                          seed_l=None):
                sums = [sb.tile([P, ntile], f32, tag=f"sums{q}", name=f"sums{q}")
                        for q in range(nfc)]
                sumsqs = [sb.tile([P, ntile], f32, tag=f"sumsq{q}", name=f"sumsq{q}")
                          for q in range(nfc)]
                for nt in range(ntile):
                    ns, ne = nt * 512, min((nt + 1) * 512, nown)
                    nn = ne - ns
                    aggT = (sb.tile([in_f, 512], f16, tag="aggT", name="aggT")
                            if width == in_f else None)
                    aggTq = ([sb.tile([P, 512], f16, tag=f"aggT{q}", name=f"aggT{q}")
                              for q in range(fi_chunks)] if width > in_f else None)
                    pair_tiles = []
                    for pr in (2 * nt, 2 * nt + 1):
                        if pr * 2 < nblk:
                            pair_tiles += aggregate_pair(
                                pr, tabA, tabB, width, "sb", row_elems, dt,
                                selftab=selftab, seed_l=seed_l)
                    for bi, b in enumerate(range(nt * 4, min(nt * 4 + 4, nblk))):
                        asb = pair_tiles[bi]
                        tp = psB.tile([P, 512], f16, tag="tp")
                        if width == in_f:
                            nc.tensor.matmul(out=tp[:width, bi * P:(bi + 1) * P],
                                             lhsT=asb[:], rhs=ident[:],
                                             is_transpose=True)
                            nc.vector.tensor_copy(out=aggT[:width, bi * P:(bi + 1) * P],
                                                  in_=tp[:width, bi * P:(bi + 1) * P])
                        else:
                            for q in range(fi_chunks):
                                nc.tensor.matmul(out=tp[:, q * P:(q + 1) * P],
                                                 lhsT=asb[:, q * P:(q + 1) * P],
                                                 rhs=ident[:], is_transpose=True)
                                if (bi + q) % 2 == 0:
                                    nc.scalar.activation(
                                        out=aggTq[q][:, bi * P:(bi + 1) * P],
                                        in_=tp[:, q * P:(q + 1) * P],
                                        func=mybir.ActivationFunctionType.Copy)
                                else:
                                    nc.vector.tensor_copy(
                                        out=aggTq[q][:, bi * P:(bi + 1) * P],
                                        in_=tp[:, q * P:(q + 1) * P])
                    for fo in range(nfc):
                        dps = psC.tile([P, 512], f32, tag="dense")
                        nmm = fi_chunks if root_pre else 2 * fi_chunks
                        mm = 0
                        for q in range(fi_chunks):
                            rhs_agg = (aggT[:width, :nn] if width == in_f
                                       else aggTq[q][:, :nn])
                            nc.tensor.matmul(out=dps[:, :nn],
                                             lhsT=W[("l", l, q)][:, fo * P:(fo + 1) * P],
                                             rhs=rhs_agg, start=(mm == 0),
                                             stop=(mm == nmm - 1))
                            mm += 1
                            if not root_pre:
                                nc.tensor.matmul(out=dps[:, :nn],
                                                 lhsT=W[("r", l, q)][:, fo * P:(fo + 1) * P],
                                                 rhs=rhs_for_fi(q, ns, ne),
                                                 start=False, stop=(mm == nmm - 1))
                                mm += 1
                        if root_pre:
                            nc.vector.scalar_tensor_tensor(
                                out=preBN[fo][:, ns:ne], in0=dps[:, :nn],
                                scalar=1.0, in1=preBN[fo][:, ns:ne],
                                op0=mybir.AluOpType.mult,
                                op1=mybir.AluOpType.add,
                                accum_out=sums[fo][:, nt:nt + 1])
                            scr = sm.tile([P, 512], f16, tag="scr", name="scr")
                            nc.scalar.activation(
                                out=scr[:, :nn], in_=preBN[fo][:, ns:ne],
                                func=mybir.ActivationFunctionType.Square,
                                accum_out=sumsqs[fo][:, nt:nt + 1])
                        else:
                            scr = sm.tile([P, 512], f16, tag="scr", name="scr")
                            nc.scalar.activation(
                                out=scr[:, :nn], in_=dps[:, :nn],
                                func=mybir.ActivationFunctionType.Square,
                                accum_out=sumsqs[fo][:, nt:nt + 1])
                            nc.vector.tensor_scalar(
                                out=preBN[fo][:, ns:ne], in0=dps[:, :nn],
                                scalar1=1.0, scalar2=None,
                                op0=mybir.AluOpType.mult,
                                op1=mybir.AluOpType.add,
                                accum_out=sums[fo][:, nt:nt + 1])
                # ---- BN statistics + cross-core allreduce ----
                pack = sb.tile([P, 8], f32, tag="pack")
                for q in range(nfc):
                    nc.vector.reduce_sum(out=pack[:, 2 * q:2 * q + 1],
                                         in_=sums[q][:], axis=mybir.AxisListType.X)
                    nc.vector.reduce_sum(out=pack[:, 2 * q + 1:2 * q + 2],
                                         in_=sumsqs[q][:], axis=mybir.AxisListType.X)
                nc.sync.dma_start(out=st_in[l][:, :], in_=pack[:])
                nc.gpsimd.collective_compute(
                    "AllReduce", mybir.AluOpType.add, replica_groups=rg,
                    ins=[st_in[l][:, :]], outs=[st_out[l][:, :]],
                )
                red = sb.tile([P, 8], f32, tag="red")
                nc.sync.dma_start(out=red[:], in_=st_out[l][:, :])
                scale = sb.tile([P, nfc], f32, tag="scale")
                shift = sb.tile([P, nfc], f32, tag="shift")
                inv_n = 1.0 / float(n_nodes)
                for q in range(nfc):
                    mu = sb.tile([P, 1], f32, tag="mu")
                    var = sb.tile([P, 1], f32, tag="var")
                    nc.vector.tensor_scalar(out=mu[:], in0=red[:, 2 * q:2 * q + 1],
                                            scalar1=inv_n, scalar2=None,
                                            op0=mybir.AluOpType.mult)
                    nc.vector.tensor_scalar(out=var[:], in0=red[:, 2 * q + 1:2 * q + 2],
                                            scalar1=inv_n, scalar2=None,
                                            op0=mybir.AluOpType.mult)
                    musq = sb.tile([P, 1], f32, tag="musq")
                    nc.vector.tensor_tensor(out=musq[:], in0=mu[:], in1=mu[:],
                                            op=mybir.AluOpType.mult)
                    nc.vector.tensor_tensor(out=var[:], in0=var[:], in1=musq[:],
                                            op=mybir.AluOpType.subtract)
                    nc.vector.tensor_scalar(out=var[:], in0=var[:], scalar1=EPS,
                                            scalar2=None, op0=mybir.AluOpType.add)
                    nc.vector.reciprocal(out=var[:], in_=var[:])
                    rs = sb.tile([P, 1], f32, tag="rs")
                    nc.scalar.activation(out=rs[:], in_=var[:],
                                         func=mybir.ActivationFunctionType.Sqrt)
                    nc.vector.tensor_tensor(out=scale[:, q:q + 1], in0=rs[:],
                                            in1=gb[("g", l)][:, q:q + 1],
                                            op=mybir.AluOpType.mult)
                    nc.vector.tensor_tensor(out=musq[:], in0=mu[:],
                                            in1=scale[:, q:q + 1],
                                            op=mybir.AluOpType.mult)
                    nc.vector.tensor_tensor(out=shift[:, q:q + 1],
                                            in0=gb[("b", l)][:, q:q + 1], in1=musq[:],
                                            op=mybir.AluOpType.subtract)
                # ---- BN apply + ReLU -> hT ----
                for q in range(nfc):
                    nc.scalar.activation(
                        out=hT[q][:, 0:nown], in_=preBN[q][:, 0:nown],
                        func=mybir.ActivationFunctionType.Relu,
                        bias=shift[:, q:q + 1], scale=scale[:, q:q + 1],
                    )
                if l == 3:
                    return
                # ---- rows + sub-AllGathers (A half first) ----
                def rows_range(b_lo, b_hi):
                    for b2 in range(b_lo, b_hi, 2):
                        bl2 = [b for b in (b2, b2 + 1) if b < b_hi]
                        w2 = len(bl2) * hid
                        tpr = psB.tile([P, 1024], f16, tag="tp")
                        for bi, b in enumerate(bl2):
                            for q in range(nfc):
                                nc.tensor.matmul(
                                    out=tpr[:, bi * hid + q * P:bi * hid + (q + 1) * P],
                                    lhsT=hT[q][:, b * P:(b + 1) * P],
                                    rhs=ident[:], is_transpose=True)
                        rows = rp.tile([P, 1024], TAB_DT[l], tag="rows")
                        if (b2 // 2) % 2 == 0:
                            nc.vector.tensor_copy(out=rows[:, :w2], in_=tpr[:, :w2])
                        else:
                            nc.scalar.activation(
                                out=rows[:, :w2], in_=tpr[:, :w2],
                                func=mybir.ActivationFunctionType.Copy)
                        for bi, b in enumerate(bl2):
                            ns, ne = b * P, min((b + 1) * P, nown)
                            nc.sync.dma_start(
                                out=h_own[l][ns:ne, :],
                                in_=rows[:ne - ns, bi * hid:bi * hid + hid])
                rows_range(0, HALF_BLK)
                nc.gpsimd.collective_compute(
                    "AllGather", mybir.AluOpType.bypass, replica_groups=rg,
                    ins=[h_own[l][0:HALF, :]], outs=[h_gA[l][:, :]],
                )
                rows_range(HALF_BLK, nblk)
                nc.gpsimd.collective_compute(
                    "AllGather", mybir.AluOpType.bypass, replica_groups=rg,
                    ins=[h_own[l][HALF:nown, :]], outs=[h_gB[l][:, :]],
                )
                # self partials for the next layer (runs in the CC window)
                self_pipeline(l + 1, h_own[l][:, :], hid, hid, TAB_DT[l])
                # next layer's root term (PE overlaps the AllGathers)
                nl = l + 1
                for nt2 in range(ntile):
                    ns2, ne2 = nt2 * 512, min((nt2 + 1) * 512, nown)
                    nn2 = ne2 - ns2
                    for fo in range(nfc):
                        rps = psC.tile([P, 512], f32, tag="dense")
                        for q in range(nfc):
                            nc.tensor.matmul(
                                out=rps[:, :nn2],
                                lhsT=W[("r", nl, q)][:, fo * P:(fo + 1) * P],
                                rhs=hT[q][:, ns2:ne2],
                                start=(q == 0), stop=(q == nfc - 1))
                        if (nt2 + fo) % 2 == 0:
                            nc.vector.tensor_copy(out=preBN[fo][:, ns2:ne2],
                                                  in_=rps[:, :nn2])
                        else:
                            nc.scalar.activation(
                                out=preBN[fo][:, ns2:ne2], in_=rps[:, :nn2],
                                func=mybir.ActivationFunctionType.Copy)

            # ================= layer 1 =================
            def xT_rhs(q, ns, ne):
                xt = sm.tile([in_f, 512], f16, tag="xTt", name="xTt")
                nc.sync.dma_start(out=xt[:, :ne - ns], in_=xT[:, ns:ne])
                return xt[:, :ne - ns]
            layer_123(1, xA_d[:, :], xB_d[:, :], 1, xT_rhs, in_f, 128, f16,
                      selftab=xself_d[:, :])
            # ================= layers 2,3 =================
            for l in (2, 3):
                layer_123(l, h_gA[l - 1][:, :], h_gB[l - 1][:, :], nfc,
                          None, hid, hid, TAB_DT[l - 1], root_pre=True,
                          seed_l=l)
            # ================= layer 4 =================
            for nt in range(ntile):
                ns, ne = nt * 512, min((nt + 1) * 512, nown)
                nn = ne - ns
                yps = psC.tile([P, 512], f32, tag="dense")
                for q in range(nfc):
                    nc.tensor.matmul(out=yps[:out_f, :nn],
                                     lhsT=W[("l", 4, q)][:, :out_f],
                                     rhs=hT[q][:, ns:ne],
                                     start=(q == 0), stop=(q == nfc - 1))
                ysb = sb.tile([P, 512], f16, tag="ysb")
                if nt % 2 == 0:
                    nc.scalar.activation(out=ysb[:out_f, :nn], in_=yps[:out_f, :nn],
                                         func=mybir.ActivationFunctionType.Copy)
                else:
                    nc.vector.tensor_copy(out=ysb[:out_f, :nn], in_=yps[:out_f, :nn])
                for bi in range((nn + P - 1) // P):
                    b0 = bi * P
                    b1 = min(b0 + P, nn)
                    tpy = psB.tile([P, 512], f16, tag="tp")
                    nc.tensor.matmul(out=tpy[:b1 - b0, :out_f],
                                     lhsT=ysb[:out_f, b0:b1],
                                     rhs=ident[:out_f, :out_f],
                                     is_transpose=True)
                    yr = sb.tile([P, 128], f16, tag="yrows")
                    # pad cols out_f..128 are never read by the agg matmuls
                    if bi % 2 == 0:
                        nc.vector.tensor_copy(out=yr[:b1 - b0, :out_f],
                                              in_=tpy[:b1 - b0, :out_f])
                    else:
                        nc.scalar.activation(out=yr[:b1 - b0, :out_f],
                                             in_=tpy[:b1 - b0, :out_f],
                                             func=mybir.ActivationFunctionType.Copy)
                    nc.sync.dma_start(out=y_own[ns + b0:ns + b1, :out_f],
                                      in_=yr[:b1 - b0, :out_f])
            nc.gpsimd.collective_compute(
                "AllGather", mybir.AluOpType.bypass, replica_groups=rg,
                ins=[y_own[0:HALF, :]], outs=[y_gA[:, :]],
            )
            nc.gpsimd.collective_compute(
                "AllGather", mybir.AluOpType.bypass, replica_groups=rg,
                ins=[y_own[HALF:nown, :]], outs=[y_gB[:, :]],
            )
            self_pipeline(4, y_own[:, :], out_f, 128, f16)
            # root term h3 @ Wr4 -> preBN[0]
            for nt in range(ntile):
                ns, ne = nt * 512, min((nt + 1) * 512, nown)
                nn = ne - ns
                rps4 = psC.tile([P, 512], f32, tag="dense")
                for q in range(nfc):
                    nc.tensor.matmul(out=rps4[:out_f, :nn],
                                     lhsT=W[("r", 4, q)][:, :out_f],
                                     rhs=hT[q][:, ns:ne],
                                     start=(q == 0), stop=(q == nfc - 1))
                if nt % 2 == 0:
                    nc.vector.tensor_copy(out=preBN[0][:out_f, ns:ne],
                                          in_=rps4[:out_f, :nn])
                else:
                    nc.scalar.activation(out=preBN[0][:out_f, ns:ne],
                                         in_=rps4[:out_f, :nn],
                                         func=mybir.ActivationFunctionType.Copy)
            # final: out = mean-agg(y) + h3 @ Wr4 + bl4
            for nt in range(ntile):
                ns, ne = nt * 512, min((nt + 1) * 512, nown)
                nn = ne - ns
                agg4T = sb.tile([P, 512], f16, tag="agg4T")
                pair_tiles4 = []
                for pr in (2 * nt, 2 * nt + 1):
                    if pr * 2 < nblk:
                        pair_tiles4 += aggregate_pair(pr, y_gA[:, :],
                                                      y_gB[:, :],
                                                      out_f, "4", 128, f16,
                                                      seed_l=4)
                for bi, b in enumerate(range(nt * 4, min(nt * 4 + 4, nblk))):
                    asb = pair_tiles4[bi]
                    tp = psB.tile([P, 512], f16, tag="tp")
                    nc.tensor.matmul(out=tp[:out_f, bi * P:(bi + 1) * P],
                                     lhsT=asb[:], rhs=ident[:], is_transpose=True)
                    if bi % 2 == 0:
                        nc.scalar.activation(out=agg4T[:out_f, bi * P:(bi + 1) * P],
                                             in_=tp[:out_f, bi * P:(bi + 1) * P],
                                             func=mybir.ActivationFunctionType.Copy)
                    else:
                        nc.vector.tensor_copy(out=agg4T[:out_f, bi * P:(bi + 1) * P],
                                              in_=tp[:out_f, bi * P:(bi + 1) * P])
                osb = sb.tile([P, 512], f32, tag="osb")
                nc.vector.scalar_tensor_tensor(
                    out=osb[:out_f, :nn], in0=preBN[0][:out_f, ns:ne],
                    scalar=bl4_t[:out_f, 0:1], in1=agg4T[:out_f, :nn],
                    op0=mybir.AluOpType.add, op1=mybir.AluOpType.add)
                for bi in range((nn + P - 1) // P):
                    b0, b1 = bi * P, min(bi * P + P, nn)
                    tpo = psB.tile([P, 512], f32, tag="tp")
                    nc.tensor.matmul(out=tpo[:b1 - b0, :out_f],
                                     lhsT=osb[:out_f, b0:b1],
                                     rhs=ident32[:out_f, :out_f],
                                     is_transpose=True)
                    orow = sb.tile([P, out_f], f32, tag="orow")
                    if bi % 2 == 0:
                        nc.vector.tensor_copy(out=orow[:b1 - b0, :],
                                              in_=tpo[:b1 - b0, :out_f])
                    else:
                        nc.scalar.activation(out=orow[:b1 - b0, :],
                                             in_=tpo[:b1 - b0, :out_f],
                                             func=mybir.ActivationFunctionType.Copy)
                    nc.sync.dma_start(out=out_d[ns + b0:ns + b1, :],
                                      in_=orow[:b1 - b0, :])
    return nc


def _execute(nc, in_maps):
    from concourse.bass_utils import run_bass_kernel_spmd
    res = run_bass_kernel_spmd(nc, in_maps, list(range(NCORES)))
    return [res.results[c] for c in range(NCORES)]


def _balance_perm(n_nodes, dst):
    """Permutation new->old assigning nodes to (core, block) bins so the
    per-block-position degree sums align across cores (kills most of the
    cross-core kmax padding in the gather plans). Snake-deal by degree."""
    nown = n_nodes // NCORES
    nblk = (nown + P - 1) // P
    last_cap = nown - (nblk - 1) * P
    caps = np.full((NCORES, nblk), P, np.int64)
    caps[:, nblk - 1] = last_cap
    deg = np.bincount(dst, minlength=n_nodes)
    order = np.argsort(-deg, kind="stable")
    bins = [[] for _ in range(NCORES * nblk)]
    flat_caps = caps.reshape(-1)
    active = list(range(NCORES * nblk))
    i = 0
    fwd = True
    while i < n_nodes:
        seq = active if fwd else active[::-1]
        for b in seq:
            if i >= n_nodes:
                break
            bins[b].append(order[i])
            i += 1
        fwd = not fwd
        active = [b for b in active if len(bins[b]) < flat_caps[b]]
    perm = np.empty(n_nodes, np.int64)
    k = 0
    for c in range(NCORES):
        for b in range(nblk):
            members = bins[c * nblk + b]
            perm[k:k + len(members)] = members
            k += len(members)
    return perm


def kernel(**inputs):
    x = np.asarray(inputs["x"], np.float32)
    edge_index = np.asarray(inputs["edge_index"])
    _dst0 = np.asarray(edge_index[1]).astype(np.int64)
    perm = _balance_perm(x.shape[0], _dst0)
    inv = np.empty_like(perm)
    inv[perm] = np.arange(perm.size)
    x = x[perm]
    edge_index = inv[np.asarray(edge_index).astype(np.int64)]
    n_nodes, in_f = x.shape
    hid = inputs["Wl2"].shape[0]
    out_f = inputs["Wl4"].shape[1]
    nown = n_nodes // NCORES
    nhB = nown - HALF

    src = np.asarray(edge_index[0]).astype(np.int64)
    dst = np.asarray(edge_index[1]).astype(np.int64)
    deg = np.bincount(dst, minlength=n_nodes).astype(np.float32)
    deginv = (1.0 / np.maximum(deg, 1.0)).astype(np.float32)

    plans = _finalize([Plan(n_nodes, src, dst, c) for c in range(NCORES)])
    pl = plans[0]
    print(f"[kernel] self chunks {pl.stot} remote chunks {pl.rtot}", flush=True)

    import time as _time
    _t0 = _time.perf_counter()
    nc = build_program(n_nodes, in_f, hid, out_f, pl)
    print(f"[kernel] program built in {_time.perf_counter() - _t0:.1f}s", flush=True)
    _t0 = _time.perf_counter()
    nc.compile()
    print(f"[kernel] bacc compile in {_time.perf_counter() - _t0:.1f}s", flush=True)

    xr = np.zeros((n_nodes, 128), np.float16)
    xr[:, :in_f] = x.astype(np.float16)
    xA = np.concatenate([xr[c * nown:c * nown + HALF] for c in range(NCORES)])
    xB = np.concatenate([xr[c * nown + HALF:(c + 1) * nown] for c in range(NCORES)])
    nblk = pl.nblk
    pad_n = nblk * P

    in_maps = []
    for c, p in enumerate(plans):
        xTc = np.zeros((in_f, pad_n), np.float16)
        xTc[:, :nown] = x[c * nown:(c + 1) * nown].T.astype(np.float16)
        dg = np.zeros(pad_n, np.float32)
        dg[:nown] = deginv[c * nown:(c + 1) * nown]
        im = {
            "xT": xTc, "xself": xr[c * nown:(c + 1) * nown],
            "xA": xA, "xB": xB,
            "idxS": p.idxS if p.idxS.size else np.zeros((P, 8), np.int16),
            "dstS": p.dstS,
            "idxR": p.idxR if p.idxR.size else np.zeros((P, 8), np.int16),
            "dstR": p.dstR,
            "deginv": dg,
            "bl4": np.asarray(inputs["bl4"], np.float32),
        }
        for l in (1, 2, 3, 4):
            im[f"Wl{l}"] = np.asarray(inputs[f"Wl{l}"], np.float16)
            im[f"Wr{l}"] = np.asarray(inputs[f"Wr{l}"], np.float16)
        for l in (1, 2, 3):
            im[f"g{l}"] = np.asarray(inputs[f"g{l}"], np.float32)
            im[f"b{l}"] = np.asarray(inputs[f"b{l}"], np.float32)
        in_maps.append(im)

    global LAST_BUILD
    LAST_BUILD = (nc, in_maps)
    results = _execute(nc, in_maps)
    out = np.concatenate([results[c]["out"] for c in range(NCORES)], axis=0)
    out_full = np.empty_like(out)
    out_full[perm] = out
    return out_full.astype(np.float32)


# revision 19
# speedup vs baseline: 1.0612x; 1.0612x over previous
"""DeepGraphSAGE (4x SAGEConv + BN/ReLU) on 8 Trainium2 NeuronCores.

Sharding: nodes partitioned across 8 cores (6250 dst nodes each). Per layer:
  - mean-aggregate neighbor features via dma_gather + one-hot selection
    matmuls accumulating in PSUM (S built on-chip via iota/is_equal).
  - edges 3-way split by source: SELF (own rows, gathered from local h_own
    during the collective window, pre-aggregated into DRAM partials),
    REMOTE-A (first HALF local rows of each owner) and REMOTE-B (rest).
    The hidden-state exchange is two sub-AllGathers (A then B) so remote-A
    gathers start as soon as the first half arrives; all index spaces fit
    int16 without a base split.
  - dense transforms in transposed layout (features on partitions)
  - BatchNorm stats via accumulated sums + tiny cross-core AllReduce
  - next layer's root term precomputed on PE during the collectives.
Hidden tables travel fp8e3 (E3M4); weights/root f16; accumulation fp32.
"""
import sys
import numpy as np

for p in ("/opt/trn_rl_repo",):
    if p not in sys.path:
        sys.path.append(p)

import concourse.bass as bass
import concourse.bacc as bacc
import concourse.mybir as mybir
from concourse.tile import TileContext
from concourse.masks import make_identity

f32 = mybir.dt.float32
f16 = mybir.dt.float16
fp8 = mybir.dt.float8e3
i16 = mybir.dt.int16

NCORES = 8
P = 128
HALF_BLK = 24
HALF = HALF_BLK * P            # 3072 local rows in the A half
EPS = 1e-5
LAST_BUILD = None
TAB_DT = {1: fp8, 2: fp8}


# ---------------------------------------------------------------- host prep
class Plan:
    """Per-core gather plan: self / remote-A / remote-B chunk streams."""

    def __init__(self, n_nodes, src, dst, core):
        nown = n_nodes // NCORES
        self.n_own = nown
        self.nblk = (nown + P - 1) // P
        self.npair = (self.nblk + 1) // 2
        nhB = nown - HALF
        lo = core * nown
        m = (dst >= lo) & (dst < lo + nown)
        es = src[m].astype(np.int64)
        ed = (dst[m] - lo).astype(np.int64)
        order = np.argsort(ed, kind="stable")
        es, ed = es[order], ed[order]
        bounds = np.searchsorted(ed, np.arange(0, self.nblk + 1) * P)
        owner = es // nown
        lr = es % nown
        grp = np.where(owner == core, 0, np.where(lr < HALF, 1, 2))
        gidx = np.where(grp == 0, lr,
                        np.where(grp == 1, owner * HALF + lr,
                                 owner * nhB + (lr - HALF)))
        # sitems[pr][bi] / ritems[pr][gi][bi] = (idx_flat[k*P], dst[P,k])
        self.sitems, self.ritems = [], []
        for pr in range(self.npair):
            blocks = [b for b in (2 * pr, 2 * pr + 1) if b < self.nblk]
            self.sitems.append(
                [self._chunk(es, ed, bounds, b, grp, gidx, 0) for b in blocks])
            self.ritems.append(
                [[self._chunk(es, ed, bounds, b, grp, gidx, gid)
                  for b in blocks] for gid in (1, 2)])

    @staticmethod
    def _chunk(es, ed, bounds, b, grp, gidx, gid):
        e0, e1 = bounds[b], bounds[b + 1]
        msel = grp[e0:e1] == gid
        gs = gidx[e0:e1][msel]
        gd = ed[e0:e1][msel] - b * P
        k = (len(gs) + P - 1) // P
        iv = np.zeros(k * P, np.int16)
        iv[:len(gs)] = gs.astype(np.int16)
        dv = np.full((P, k), -1, np.int16)
        if k:
            dpad = np.full(k * P, -1, np.int64)
            dpad[:len(gd)] = gd
            dv = dpad.reshape(k, P).T.astype(np.int16).copy()
        return iv, dv


def _pack(idx_vals, dst_vals):
    iv = np.concatenate(idx_vals) if idx_vals else np.zeros(0, np.int16)
    w = iv.reshape(-1, 16).T if iv.size else np.zeros((16, 0), np.int16)
    idx16 = np.tile(w, (8, 1)).copy()
    dst16 = (np.concatenate(dst_vals, axis=1).copy()
             if dst_vals else np.zeros((P, 1), np.int16))
    return idx16, dst16


def _finalize(plans):
    """Cross-core pad chunk counts; build packed idx/dst tables + call lists."""
    npair = plans[0].npair

    def padded(item, km):
        iv, dv = item
        k = dv.shape[1] if dv.size else 0
        iv2 = np.zeros(km * P, np.int16)
        iv2[:k * P] = iv
        dv2 = np.full((P, km), -1, np.int16)
        if k:
            dv2[:, :k] = dv
        return iv2, dv2

    kmax_s = [[max(len(p.sitems[pr][bi][0]) // P for p in plans)
               for bi in range(len(plans[0].sitems[pr]))]
              for pr in range(npair)]
    kmax_r = [[[max(len(p.ritems[pr][gi][bi][0]) // P for p in plans)
                for bi in range(len(plans[0].ritems[pr][gi]))]
               for gi in range(2)] for pr in range(npair)]
    for p in plans:
        sidx, sdst, ridx, rdst = [], [], [], []
        for pr in range(npair):
            for bi, km in enumerate(kmax_s[pr]):
                iv, dv = padded(p.sitems[pr][bi], km)
                if km:
                    sidx.append(iv)
                    sdst.append(dv)
            for gi in range(2):
                for bi, km in enumerate(kmax_r[pr][gi]):
                    iv, dv = padded(p.ritems[pr][gi][bi], km)
                    if km:
                        ridx.append(iv)
                        rdst.append(dv)
        p.idxS, p.dstS = _pack(sidx, sdst)
        p.idxR, p.dstR = _pack(ridx, rdst)
    pl = plans[0]
    pl.scalls = [list(kmax_s[pr]) for pr in range(npair)]
    pl.rcalls = [[(gi + 1, list(kmax_r[pr][gi])) for gi in range(2)]
                 for pr in range(npair)]
    pl.schoff, pl.rchoff = [], []
    so = ro = 0
    for pr in range(npair):
        pl.schoff.append(so)
        pl.rchoff.append(ro)
        so += sum(kmax_s[pr])
        ro += sum(sum(ks) for _, ks in pl.rcalls[pr])
    pl.stot, pl.rtot = so, ro
    return plans


# ---------------------------------------------------------------- program
def build_program(n_nodes, in_f, hid, out_f, pl):
    nown = pl.n_own
    nblk = pl.nblk
    pad_n = nblk * P
    ntile = (nown + 511) // 512
    nfc = hid // P
    npair = pl.npair
    nhB = nown - HALF
    dims = [(in_f, hid), (hid, hid), (hid, hid), (hid, out_f)]

    nc = bacc.Bacc("TRN2", target_bir_lowering=False, debug=False,
                   num_devices=NCORES, num_swdge_queues=4)

    # ---- I/O ----
    xT = nc.dram_tensor("xT", [in_f, pad_n], f16, kind="ExternalInput")
    xself_d = nc.dram_tensor("xself", [nown, 128], f16, kind="ExternalInput")
    xA_d = nc.dram_tensor("xA", [NCORES * HALF, 128], f16, kind="ExternalInput")
    xB_d = nc.dram_tensor("xB", [NCORES * nhB, 128], f16, kind="ExternalInput")
    idxS_d = nc.dram_tensor("idxS", [P, max(pl.stot * 8, 8)], i16, kind="ExternalInput")
    dstS_d = nc.dram_tensor("dstS", [P, max(pl.stot, 1)], i16, kind="ExternalInput")
    idxR_d = nc.dram_tensor("idxR", [P, max(pl.rtot * 8, 8)], i16, kind="ExternalInput")
    dstR_d = nc.dram_tensor("dstR", [P, max(pl.rtot, 1)], i16, kind="ExternalInput")
    deginv_d = nc.dram_tensor("deginv", [pad_n], f32, kind="ExternalInput")
    wl_d, wr_d, g_d, b_d = {}, {}, {}, {}
    for l, (fi, fo) in enumerate(dims, start=1):
        wl_d[l] = nc.dram_tensor(f"Wl{l}", [fi, fo], f16, kind="ExternalInput")
        wr_d[l] = nc.dram_tensor(f"Wr{l}", [fi, fo], f16, kind="ExternalInput")
    for l in (1, 2, 3):
        g_d[l] = nc.dram_tensor(f"g{l}", [hid], f32, kind="ExternalInput")
        b_d[l] = nc.dram_tensor(f"b{l}", [hid], f32, kind="ExternalInput")
    bl4_d = nc.dram_tensor("bl4", [out_f], f32, kind="ExternalInput")
    out_d = nc.dram_tensor("out", [nown, out_f], f32, kind="ExternalOutput")

    # ---- internal DRAM ----
    h_own = {l: nc.dram_tensor(f"h{l}_own", [nown, hid], TAB_DT[l]) for l in (1, 2)}
    h_gA = {l: nc.dram_tensor(f"h{l}_gA", [NCORES * HALF, hid], TAB_DT[l],
                              addr_space="Shared") for l in (1, 2)}
    h_gB = {l: nc.dram_tensor(f"h{l}_gB", [NCORES * nhB, hid], TAB_DT[l],
                              addr_space="Shared") for l in (1, 2)}
    y_own = nc.dram_tensor("y_own", [nown, 128], f16)
    y_gA = nc.dram_tensor("y_gA", [NCORES * HALF, 128], f16, addr_space="Shared")
    y_gB = nc.dram_tensor("y_gB", [NCORES * nhB, 128], f16, addr_space="Shared")
    spart = {2: nc.dram_tensor("sp2", [pad_n, hid], f16),
             3: nc.dram_tensor("sp3", [pad_n, hid], f16),
             4: nc.dram_tensor("sp4", [pad_n, 128], f16)}
    st_in = {l: nc.dram_tensor(f"st{l}_in", [P, 8], f32) for l in (1, 2, 3)}
    st_out = {l: nc.dram_tensor(f"st{l}_out", [P, 8], f32, addr_space="Shared")
              for l in (1, 2, 3)}
    rg = [list(range(NCORES))]

    with TileContext(nc) as tc:
        with (
            tc.tile_pool(name="const", bufs=1) as cp,
            tc.tile_pool(name="sbuf", bufs=2) as sb,
            tc.tile_pool(name="small", bufs=3) as sm,
            tc.tile_pool(name="spool", bufs=2) as sp,
            tc.tile_pool(name="gpool", bufs=3) as gp,
            tc.tile_pool(name="sspool", bufs=2) as ssp,
            tc.tile_pool(name="sgpool", bufs=2) as sgp,
            tc.tile_pool(name="ppool", bufs=2) as pp,
            tc.tile_pool(name="rows", bufs=2) as rp,
            tc.tile_pool(name="psA", bufs=2, space="PSUM") as psA,
            tc.tile_pool(name="psB", bufs=2, space="PSUM") as psB,
            tc.tile_pool(name="psC", bufs=2, space="PSUM") as psC,
        ):
            ident = cp.tile([P, P], f16)
            make_identity(nc, ident[:])
            ident32 = cp.tile([P, P], f32)
            make_identity(nc, ident32[:])
            iota_t = cp.tile([P, P], i16)
            nc.gpsimd.iota(iota_t[:], pattern=[[1, P]], base=0,
                           channel_multiplier=0,
                           allow_small_or_imprecise_dtypes=True)
            deginv_t = cp.tile([P, nblk], f32)
            nc.sync.dma_start(out=deginv_t[:],
                              in_=deginv_d[:].rearrange("(b p) -> p b", p=P))
            idxSc = cp.tile([P, max(pl.stot * 8, 8)], i16)
            nc.sync.dma_start(out=idxSc[:], in_=idxS_d[:, :])
            dstSc = cp.tile([P, max(pl.stot, 1)], i16)
            nc.sync.dma_start(out=dstSc[:], in_=dstS_d[:, :])
            idxRc = cp.tile([P, max(pl.rtot * 8, 8)], i16)
            nc.sync.dma_start(out=idxRc[:], in_=idxR_d[:, :])
            dstRc = cp.tile([P, max(pl.rtot, 1)], i16)
            nc.sync.dma_start(out=dstRc[:], in_=dstR_d[:, :])
            W = {}
            for l, (fi, fo) in enumerate(dims, start=1):
                kc = (fi + P - 1) // P
                for (nm, dram) in (("l", wl_d[l]), ("r", wr_d[l])):
                    for q in range(kc):
                        r0, r1 = q * P, min((q + 1) * P, fi)
                        t = cp.tile([r1 - r0, fo], f16, tag=f"W{nm}{l}_{q}")
                        nc.sync.dma_start(out=t[:], in_=dram[r0:r1, :])
                        W[(nm, l, q)] = t
            gb = {}
            for l in (1, 2, 3):
                for nm, dram in (("g", g_d[l]), ("b", b_d[l])):
                    t = cp.tile([P, nfc], f32, tag=f"{nm}{l}")
                    nc.sync.dma_start(out=t[:], in_=dram[:].rearrange("(c p) -> p c", p=P))
                    gb[(nm, l)] = t
            bl4_t = cp.tile([P, 1], f32)
            nc.sync.dma_start(out=bl4_t[:out_f, :], in_=bl4_d[:, None])

            hT = [cp.tile([P, pad_n], f16, tag=f"hT{q}", name=f"hT{q}") for q in range(nfc)]
            preBN = [cp.tile([P, pad_n], f16, tag=f"preBN{q}", name=f"preBN{q}") for q in range(nfc)]
            if pad_n > nown:
                for q in range(nfc):
                    nc.vector.memset(hT[q][:, nown:pad_n], 0.0)

            qload = [0, 0, 0, 0]  # per-queue gathered-row balance

            def next_queue(rows):
                q = min(range(4), key=lambda i: qload[i])
                qload[q] += rows
                return q

            # ---------------- self partial pipeline ----------------
            def self_pipeline(l, table_ap, width, row_elems, dt):
                """Aggregate SELF chunks from the local table into DRAM
                partials [pad_n, width]; runs inside the collective window."""
                for pr in range(npair):
                    ks = pl.scalls[pr]
                    ktot = sum(ks)
                    ch0 = pl.schoff[pr]
                    blocks = [b for b in (2 * pr, 2 * pr + 1) if b < nblk]
                    sS = gS = None
                    if ktot:
                        sS = ssp.tile([P, ktot, P], dt, tag="Ss")
                        nc.vector.tensor_tensor(
                            out=sS[:],
                            in0=dstSc[:, ch0:ch0 + ktot].unsqueeze(2)
                                .broadcast_to([P, ktot, P]),
                            in1=iota_t[:].unsqueeze(1).broadcast_to([P, ktot, P]),
                            op=mybir.AluOpType.is_equal)
                        gS = sgp.tile([P, ktot, row_elems], dt, tag="Gs")
                        nc.gpsimd.dma_gather(
                            out_ap=gS[:], in_ap=table_ap,
                            idxs_ap=idxSc[:, ch0 * 8:(ch0 + ktot) * 8],
                            num_idxs=ktot * P, num_idxs_reg=ktot * P,
                            elem_size=row_elems, single_packet=False,
                            queue_num=next_queue(ktot * P))
                    koff = 0
                    for bi, b in enumerate(blocks):
                        k = ks[bi]
                        zsb = pp.tile([P, width], f16, tag="pout",
                                      name=f"pout{bi}")
                        if k == 0:
                            nc.vector.memset(zsb[:], 0.0)
                        else:
                            ps = psA.tile([P, 512], f32, tag=f"agg_ps{bi}",
                                          name=f"psS{bi}")
                            for n_, j in enumerate(range(koff, koff + k)):
                                nc.tensor.matmul(
                                    out=ps[:, :width], lhsT=sS[:, j, :],
                                    rhs=gS[:, j, :width],
                                    start=(n_ == 0), stop=(n_ == k - 1))
                            if b % 2 == 0:
                                nc.scalar.activation(
                                    out=zsb[:], in_=ps[:, :width],
                                    func=mybir.ActivationFunctionType.Copy)
                            else:
                                nc.vector.tensor_copy(out=zsb[:],
                                                      in_=ps[:, :width])
                        nc.sync.dma_start(
                            out=spart[l][b * P:(b + 1) * P, :width],
                            in_=zsb[:])
                        koff += k

            # ---------------- aggregation ----------------
            def aggregate_pair(pr, tabA, tabB, width, tagsfx, row_elems, dt,
                               selftab=None, seed_l=None):
                """Mean-aggregate both blocks of pair pr from remote tables,
                plus either inline self chunks (selftab) or a DRAM partial
                seed (seed_l). Returns f16 tiles per block."""
                groups = pl.rcalls[pr]
                blocks = [b for b in (2 * pr, 2 * pr + 1) if b < nblk]
                ch0 = pl.rchoff[pr]
                rktot = sum(sum(ks) for _, ks in groups)
                out_tiles = []
                stile = g = None
                if rktot:
                    stile = sp.tile([P, rktot, P], dt, tag="S")
                    nc.vector.tensor_tensor(
                        out=stile[:],
                        in0=dstRc[:, ch0:ch0 + rktot].unsqueeze(2)
                            .broadcast_to([P, rktot, P]),
                        in1=iota_t[:].unsqueeze(1).broadcast_to([P, rktot, P]),
                        op=mybir.AluOpType.is_equal)
                    g = gp.tile([P, rktot, row_elems], dt, tag="G")
                    koff = 0
                    for gid, ks in groups:
                        k = sum(ks)
                        if k == 0:
                            continue
                        src_ap = tabA if gid == 1 else tabB
                        parts = ([k] if k <= 4 else [(k + 1) // 2, k // 2])
                        po = 0
                        for kp in parts:
                            if kp == 0:
                                continue
                            o = koff + po
                            nc.gpsimd.dma_gather(
                                out_ap=g[:, o:o + kp, :], in_ap=src_ap,
                                idxs_ap=idxRc[:, (ch0 + o) * 8:(ch0 + o + kp) * 8],
                                num_idxs=kp * P, num_idxs_reg=kp * P,
                                elem_size=row_elems, single_packet=False,
                                queue_num=next_queue(kp * P))
                            po += kp
                        koff += k
                # inline self chunks (layer 1)
                sS = gS = None
                sks = pl.scalls[pr]
                sch0 = pl.schoff[pr]
                if selftab is not None and sum(sks):
                    sktot = sum(sks)
                    sS = ssp.tile([P, sktot, P], dt, tag="Ss")
                    nc.vector.tensor_tensor(
                        out=sS[:],
                        in0=dstSc[:, sch0:sch0 + sktot].unsqueeze(2)
                            .broadcast_to([P, sktot, P]),
                        in1=iota_t[:].unsqueeze(1).broadcast_to([P, sktot, P]),
                        op=mybir.AluOpType.is_equal)
                    gS = sgp.tile([P, sktot, row_elems], dt, tag="Gs")
                    nc.gpsimd.dma_gather(
                        out_ap=gS[:], in_ap=selftab,
                        idxs_ap=idxSc[:, sch0 * 8:(sch0 + sktot) * 8],
                        num_idxs=sktot * P, num_idxs_reg=sktot * P,
                        elem_size=row_elems, single_packet=False,
                        queue_num=next_queue(sktot * P))
                for bi, b in enumerate(blocks):
                    # remote chunk js of this block
                    mm_idx = []
                    koff = 0
                    for gid, ks in groups:
                        pre = 0
                        for i2, k2 in enumerate(ks):
                            if i2 == bi:
                                mm_idx += list(range(koff + pre, koff + pre + k2))
                            pre += k2
                        koff += sum(ks)
                    smm = []
                    if sS is not None:
                        pre = 0
                        for i2, k2 in enumerate(sks):
                            if i2 == bi:
                                smm = list(range(pre, pre + k2))
                            pre += k2
                    nmm = len(mm_idx) + len(smm) + (1 if seed_l else 0)
                    if nmm == 0:
                        z = sm.tile([P, width], f16, tag=f"agg{tagsfx}{bi}",
                                    name=f"aggz{bi}")
                        nc.vector.memset(z[:], 0.0)
                        out_tiles.append(z)
                        continue
                    agg_ps = psA.tile([P, 512], f32, tag=f"agg_ps{bi}",
                                      name=f"agg_ps{bi}")
                    mm = 0
                    if seed_l:
                        seed = sm.tile([P, width], f16, tag="seed",
                                       name=f"seed{bi}")
                        nc.sync.dma_start(out=seed[:],
                                          in_=spart[seed_l][b * P:(b + 1) * P, :width])
                        nc.tensor.matmul(out=agg_ps[:, :width], lhsT=ident[:],
                                         rhs=seed[:], start=True,
                                         stop=(nmm == 1))
                        mm = 1
                    for j in smm:
                        nc.tensor.matmul(out=agg_ps[:, :width],
                                         lhsT=sS[:, j, :], rhs=gS[:, j, :width],
                                         start=(mm == 0), stop=(mm == nmm - 1))
                        mm += 1
                    for j in mm_idx:
                        nc.tensor.matmul(out=agg_ps[:, :width],
                                         lhsT=stile[:, j, :], rhs=g[:, j, :width],
                                         start=(mm == 0), stop=(mm == nmm - 1))
                        mm += 1
                    asb = sm.tile([P, width], f16, tag=f"agg{tagsfx}{bi}",
                                  name=f"asb{bi}")
                    if b % 2 == 0:
                        nc.scalar.activation(
                            out=asb[:], in_=agg_ps[:, :width],
                            func=mybir.ActivationFunctionType.Copy,
                            scale=deginv_t[:, b:b + 1])
                    else:
                        nc.vector.tensor_scalar(
                            out=asb[:], in0=agg_ps[:, :width],
                            scalar1=deginv_t[:, b:b + 1], scalar2=None,
                            op0=mybir.AluOpType.mult)
                    out_tiles.append(asb)
                return out_tiles

            # ---------------- one SAGE layer with BN+ReLU ----------------
            def layer_123(l, tabA, tabB, fi_chunks, rhs_for_fi, width,
                          row_elems, dt, root_pre=False, selftab=None,
                          seed_l=None):
                sums = [sb.tile([P, ntile], f32, tag=f"sums{q}", name=f"sums{q}")
                        for q in range(nfc)]
                sumsqs = [sb.tile([P, ntile], f32, tag=f"sumsq{q}", name=f"sumsq{q}")
                          for q in range(nfc)]
                for nt in range(ntile):
                    ns, ne = nt * 512, min((nt + 1) * 512, nown)
                    nn = ne - ns
                    aggT = (sb.tile([in_f, 512], f16, tag="aggT", name="aggT")
                            if width == in_f else None)
                    aggTq = ([sb.tile([P, 512], f16, tag=f"aggT{q}", name=f"aggT{q}")
                              for q in range(fi_chunks)] if width > in_f else None)
                    pair_tiles = []
                    for pr in (2 * nt, 2 * nt + 1):
                        if pr * 2 < nblk:
                            pair_tiles += aggregate_pair(
                                pr, tabA, tabB, width, "sb", row_elems, dt,
                                selftab=selftab, seed_l=seed_l)
                    for bi, b in enumerate(range(nt * 4, min(nt * 4 + 4, nblk))):
                        asb = pair_tiles[bi]
                        tp = psB.tile([P, 512], f16, tag="tp")
                        if width == in_f:
                            nc.tensor.matmul(out=tp[:width, bi * P:(bi + 1) * P],
                                             lhsT=asb[:], rhs=ident[:],
                                             is_transpose=True)
                            nc.vector.tensor_copy(out=aggT[:width, bi * P:(bi + 1) * P],
                                                  in_=tp[:width, bi * P:(bi + 1) * P])
                        else:
                            for q in range(fi_chunks):
                                nc.tensor.matmul(out=tp[:, q * P:(q + 1) * P],
                                                 lhsT=asb[:, q * P:(q + 1) * P],
                                                 rhs=ident[:], is_transpose=True)
                                if (bi + q) % 2 == 0:
                                    nc.scalar.activation(
                                        out=aggTq[q][:, bi * P:(bi + 1) * P],
                                        in_=tp[:, q * P:(q + 1) * P],
                                        func=mybir.ActivationFunctionType.Copy)
                                else:
                                    nc.vector.tensor_copy(
                                        out=aggTq[q][:, bi * P:(bi + 1) * P],
                                        in_=tp[:, q * P:(q + 1) * P])
                    for fo in range(nfc):
                        dps = psC.tile([P, 512], f32, tag="dense")
                        nmm = fi_chunks if root_pre else 2 * fi_chunks
                        mm = 0
                        for q in range(fi_chunks):
                            rhs_agg = (aggT[:width, :nn] if width == in_f
                                       else aggTq[q][:, :nn])
                            nc.tensor.matmul(out=dps[:, :nn],
                                             lhsT=W[("l", l, q)][:, fo * P:(fo + 1) * P],
                                             rhs=rhs_agg, start=(mm == 0),
                                             stop=(mm == nmm - 1))
                            mm += 1
                            if not root_pre:
                                nc.tensor.matmul(out=dps[:, :nn],
                                                 lhsT=W[("r", l, q)][:, fo * P:(fo + 1) * P],
                                                 rhs=rhs_for_fi(q, ns, ne),
                                                 start=False, stop=(mm == nmm - 1))
                                mm += 1
                        if root_pre:
                            nc.vector.scalar_tensor_tensor(
                                out=preBN[fo][:, ns:ne], in0=dps[:, :nn],
                                scalar=1.0, in1=preBN[fo][:, ns:ne],
                                op0=mybir.AluOpType.mult,
                                op1=mybir.AluOpType.add,
                                accum_out=sums[fo][:, nt:nt + 1])
                            scr = sm.tile([P, 512], f16, tag="scr", name="scr")
                            nc.scalar.activation(
                                out=scr[:, :nn], in_=preBN[fo][:, ns:ne],
                                func=mybir.ActivationFunctionType.Square,
                                accum_out=sumsqs[fo][:, nt:nt + 1])
                        else:
                            scr = sm.tile([P, 512], f16, tag="scr", name="scr")
                            nc.scalar.activation(
                                out=scr[:, :nn], in_=dps[:, :nn],
                                func=mybir.ActivationFunctionType.Square,
                                accum_out=sumsqs[fo][:, nt:nt + 1])
                            nc.vector.tensor_scalar(
                                out=preBN[fo][:, ns:ne], in0=dps[:, :nn],
                                scalar1=1.0, scalar2=None,
                                op0=mybir.AluOpType.mult,
                                op1=mybir.AluOpType.add,
                                accum_out=sums[fo][:, nt:nt + 1])
                # ---- BN statistics + cross-core allreduce ----
                pack = sb.tile([P, 8], f32, tag="pack")
                for q in range(nfc):
                    nc.vector.reduce_sum(out=pack[:, 2 * q:2 * q + 1],
                                         in_=sums[q][:], axis=mybir.AxisListType.X)
                    nc.vector.reduce_sum(out=pack[:, 2 * q + 1:2 * q + 2],
                                         in_=sumsqs[q][:], axis=mybir.AxisListType.X)
                nc.sync.dma_start(out=st_in[l][:, :], in_=pack[:])
                nc.gpsimd.collective_compute(
                    "AllReduce", mybir.AluOpType.add, replica_groups=rg,
                    ins=[st_in[l][:, :]], outs=[st_out[l][:, :]],
                )
                red = sb.tile([P, 8], f32, tag="red")
                nc.sync.dma_start(out=red[:], in_=st_out[l][:, :])
                scale = sb.tile([P, nfc], f32, tag="scale")
                shift = sb.tile([P, nfc], f32, tag="shift")
                inv_n = 1.0 / float(n_nodes)
                for q in range(nfc):
                    mu = sb.tile([P, 1], f32, tag="mu")
                    var = sb.tile([P, 1], f32, tag="var")
                    nc.vector.tensor_scalar(out=mu[:], in0=red[:, 2 * q:2 * q + 1],
                                            scalar1=inv_n, scalar2=None,
                                            op0=mybir.AluOpType.mult)
                    nc.vector.tensor_scalar(out=var[:], in0=red[:, 2 * q + 1:2 * q + 2],
                                            scalar1=inv_n, scalar2=None,
                                            op0=mybir.AluOpType.mult)
                    musq = sb.tile([P, 1], f32, tag="musq")
                    nc.vector.tensor_tensor(out=musq[:], in0=mu[:], in1=mu[:],
                                            op=mybir.AluOpType.mult)
                    nc.vector.tensor_tensor(out=var[:], in0=var[:], in1=musq[:],
                                            op=mybir.AluOpType.subtract)
                    nc.vector.tensor_scalar(out=var[:], in0=var[:], scalar1=EPS,
                                            scalar2=None, op0=mybir.AluOpType.add)
                    nc.vector.reciprocal(out=var[:], in_=var[:])
                    rs = sb.tile([P, 1], f32, tag="rs")
                    nc.scalar.activation(out=rs[:], in_=var[:],
                                         func=mybir.ActivationFunctionType.Sqrt)
                    nc.vector.tensor_tensor(out=scale[:, q:q + 1], in0=rs[:],
                                            in1=gb[("g", l)][:, q:q + 1],
                                            op=mybir.AluOpType.mult)
                    nc.vector.tensor_tensor(out=musq[:], in0=mu[:],
                                            in1=scale[:, q:q + 1],
                                            op=mybir.AluOpType.mult)
                    nc.vector.tensor_tensor(out=shift[:, q:q + 1],
                                            in0=gb[("b", l)][:, q:q + 1], in1=musq[:],
                                            op=mybir.AluOpType.subtract)
                # ---- BN apply + ReLU -> hT ----
                for q in range(nfc):
                    nc.scalar.activation(
                        out=hT[q][:, 0:nown], in_=preBN[q][:, 0:nown],
                        func=mybir.ActivationFunctionType.Relu,
                        bias=shift[:, q:q + 1], scale=scale[:, q:q + 1],
                    )
                if l == 3:
                    return
                # ---- rows + sub-AllGathers (A half first) ----
                def rows_range(b_lo, b_hi):
                    for b2 in range(b_lo, b_hi, 2):
                        bl2 = [b for b in (b2, b2 + 1) if b < b_hi]
                        w2 = len(bl2) * hid
                        tpr = psB.tile([P, 1024], f16, tag="tp")
                        for bi, b in enumerate(bl2):
                            for q in range(nfc):
                                nc.tensor.matmul(
                                    out=tpr[:, bi * hid + q * P:bi * hid + (q + 1) * P],
                                    lhsT=hT[q][:, b * P:(b + 1) * P],
                                    rhs=ident[:], is_transpose=True)
                        rows = rp.tile([P, 1024], TAB_DT[l], tag="rows")
                        if (b2 // 2) % 2 == 0:
                            nc.vector.tensor_copy(out=rows[:, :w2], in_=tpr[:, :w2])
                        else:
                            nc.scalar.activation(
                                out=rows[:, :w2], in_=tpr[:, :w2],
                                func=mybir.ActivationFunctionType.Copy)
                        for bi, b in enumerate(bl2):
                            ns, ne = b * P, min((b + 1) * P, nown)
                            nc.sync.dma_start(
                                out=h_own[l][ns:ne, :],
                                in_=rows[:ne - ns, bi * hid:bi * hid + hid])
                rows_range(0, HALF_BLK)
                nc.gpsimd.collective_compute(
                    "AllGather", mybir.AluOpType.bypass, replica_groups=rg,
                    ins=[h_own[l][0:HALF, :]], outs=[h_gA[l][:, :]],
                )
                rows_range(HALF_BLK, nblk)
                nc.gpsimd.collective_compute(
                    "AllGather", mybir.AluOpType.bypass, replica_groups=rg,
                    ins=[h_own[l][HALF:nown, :]], outs=[h_gB[l][:, :]],
                )
                # self partials for the next layer (runs in the CC window)
                self_pipeline(l + 1, h_own[l][:, :], hid, hid, TAB_DT[l])
                # next layer's root term (PE overlaps the AllGathers)
                nl = l + 1
                for nt2 in range(ntile):
                    ns2, ne2 = nt2 * 512, min((nt2 + 1) * 512, nown)
                    nn2 = ne2 - ns2
                    for fo in range(nfc):
                        rps = psC.tile([P, 512], f32, tag="dense")
                        for q in range(nfc):
                            nc.tensor.matmul(
                                out=rps[:, :nn2],
                                lhsT=W[("r", nl, q)][:, fo * P:(fo + 1) * P],
                                rhs=hT[q][:, ns2:ne2],
                                start=(q == 0), stop=(q == nfc - 1))
                        if (nt2 + fo) % 2 == 0:
                            nc.vector.tensor_copy(out=preBN[fo][:, ns2:ne2],
                                                  in_=rps[:, :nn2])
                        else:
                            nc.scalar.activation(
                                out=preBN[fo][:, ns2:ne2], in_=rps[:, :nn2],
                                func=mybir.ActivationFunctionType.Copy)

            # ================= layer 1 =================
            def xT_rhs(q, ns, ne):
                xt = sm.tile([in_f, 512], f16, tag="xTt", name="xTt")
                nc.sync.dma_start(out=xt[:, :ne - ns], in_=xT[:, ns:ne])
                return xt[:, :ne - ns]
            layer_123(1, xA_d[:, :], xB_d[:, :], 1, xT_rhs, in_f, 128, f16,
                      selftab=xself_d[:, :])
            # ================= layers 2,3 =================
            for l in (2, 3):
                layer_123(l, h_gA[l - 1][:, :], h_gB[l - 1][:, :], nfc,
                          None, hid, hid, TAB_DT[l - 1], root_pre=True,
                          seed_l=l)
            # ================= layer 4 =================
            for nt in range(ntile):
                ns, ne = nt * 512, min((nt + 1) * 512, nown)
                nn = ne - ns
                yps = psC.tile([P, 512], f32, tag="dense")
                for q in range(nfc):
                    nc.tensor.matmul(out=yps[:out_f, :nn],
                                     lhsT=W[("l", 4, q)][:, :out_f],
                                     rhs=hT[q][:, ns:ne],
                                     start=(q == 0), stop=(q == nfc - 1))
                ysb = sb.tile([P, 512], f16, tag="ysb")
                if nt % 2 == 0:
                    nc.scalar.activation(out=ysb[:out_f, :nn], in_=yps[:out_f, :nn],
                                         func=mybir.ActivationFunctionType.Copy)
                else:
                    nc.vector.tensor_copy(out=ysb[:out_f, :nn], in_=yps[:out_f, :nn])
                for bi in range((nn + P - 1) // P):
                    b0 = bi * P
                    b1 = min(b0 + P, nn)
                    tpy = psB.tile([P, 512], f16, tag="tp")
                    nc.tensor.matmul(out=tpy[:b1 - b0, :out_f],
                                     lhsT=ysb[:out_f, b0:b1],
                                     rhs=ident[:out_f, :out_f],
                                     is_transpose=True)
                    yr = sb.tile([P, 128], f16, tag="yrows")
                    # pad cols out_f..128 are never read by the agg matmuls
                    if bi % 2 == 0:
                        nc.vector.tensor_copy(out=yr[:b1 - b0, :out_f],
                                              in_=tpy[:b1 - b0, :out_f])
                    else:
                        nc.scalar.activation(out=yr[:b1 - b0, :out_f],
                                             in_=tpy[:b1 - b0, :out_f],
                                             func=mybir.ActivationFunctionType.Copy)
                    nc.sync.dma_start(out=y_own[ns + b0:ns + b1, :out_f],
                                      in_=yr[:b1 - b0, :out_f])
            nc.gpsimd.collective_compute(
                "AllGather", mybir.AluOpType.bypass, replica_groups=rg,
                ins=[y_own[0:HALF, :]], outs=[y_gA[:, :]],
            )
            nc.gpsimd.collective_compute(
                "AllGather", mybir.AluOpType.bypass, replica_groups=rg,
                ins=[y_own[HALF:nown, :]], outs=[y_gB[:, :]],
            )
            self_pipeline(4, y_own[:, :], out_f, 128, f16)
            # root term h3 @ Wr4 -> preBN[0]
            for nt in range(ntile):
                ns, ne = nt * 512, min((nt + 1) * 512, nown)
                nn = ne - ns
                rps4 = psC.tile([P, 512], f32, tag="dense")
                for q in range(nfc):
                    nc.tensor.matmul(out=rps4[:out_f, :nn],
                                     lhsT=W[("r", 4, q)][:, :out_f],
                                     rhs=hT[q][:, ns:ne],
                                     start=(q == 0), stop=(q == nfc - 1))
                if nt % 2 == 0:
                    nc.vector.tensor_copy(out=preBN[0][:out_f, ns:ne],
                                          in_=rps4[:out_f, :nn])
                else:
                    nc.scalar.activation(out=preBN[0][:out_f, ns:ne],
                                         in_=rps4[:out_f, :nn],
                                         func=mybir.ActivationFunctionType.Copy)
            # final: out = mean-agg(y) + h3 @ Wr4 + bl4
            for nt in range(ntile):
                ns, ne = nt * 512, min((nt + 1) * 512, nown)
                nn = ne - ns
                agg4T = sb.tile([P, 512], f16, tag="agg4T")
                pair_tiles4 = []
                for pr in (2 * nt, 2 * nt + 1):
                    if pr * 2 < nblk:
                        pair_tiles4 += aggregate_pair(pr, y_gA[:, :],
                                                      y_gB[:, :],
                                                      out_f, "4", 128, f16,
                                                      seed_l=4)
                for bi, b in enumerate(range(nt * 4, min(nt * 4 + 4, nblk))):
                    asb = pair_tiles4[bi]
                    tp = psB.tile([P, 512], f16, tag="tp")
                    nc.tensor.matmul(out=tp[:out_f, bi * P:(bi + 1) * P],
                                     lhsT=asb[:], rhs=ident[:], is_transpose=True)
                    if bi % 2 == 0:
                        nc.scalar.activation(out=agg4T[:out_f, bi * P:(bi + 1) * P],
                                             in_=tp[:out_f, bi * P:(bi + 1) * P],
                                             func=mybir.ActivationFunctionType.Copy)
                    else:
                        nc.vector.tensor_copy(out=agg4T[:out_f, bi * P:(bi + 1) * P],
                                              in_=tp[:out_f, bi * P:(bi + 1) * P])
                osb = sb.tile([P, 512], f32, tag="osb")
                nc.vector.scalar_tensor_tensor(
                    out=osb[:out_f, :nn], in0=preBN[0][:out_f, ns:ne],
                    scalar=bl4_t[:out_f, 0:1], in1=agg4T[:out_f, :nn],
                    op0=mybir.AluOpType.add, op1=mybir.AluOpType.add)
                for bi in range((nn + P - 1) // P):
                    b0, b1 = bi * P, min(bi * P + P, nn)
                    tpo = psB.tile([P, 512], f32, tag="tp")
                    nc.tensor.matmul(out=tpo[:b1 - b0, :out_f],
                                     lhsT=osb[:out_f, b0:b1],
                                     rhs=ident32[:out_f, :out_f],
                                     is_transpose=True)
                    orow = sb.tile([P, out_f], f32, tag="orow")
                    if bi % 2 == 0:
                        nc.vector.tensor_copy(out=orow[:b1 - b0, :],
                                              in_=tpo[:b1 - b0, :out_f])
                    else:
                        nc.scalar.activation(out=orow[:b1 - b0, :],
                                             in_=tpo[:b1 - b0, :out_f],
                                             func=mybir.ActivationFunctionType.Copy)
                    nc.sync.dma_start(out=out_d[ns + b0:ns + b1, :],
                                      in_=orow[:b1 - b0, :])
    return nc


def _execute(nc, in_maps):
    from concourse.bass_utils import run_bass_kernel_spmd
    res = run_bass_kernel_spmd(nc, in_maps, list(range(NCORES)))
    return [res.results[c] for c in range(NCORES)]


def _balance_perm(n_nodes, dst):
    """Permutation new->old assigning nodes to (core, block) bins so the
    per-block-position degree sums align across cores (kills most of the
    cross-core kmax padding in the gather plans). Snake-deal by degree."""
    nown = n_nodes // NCORES
    nblk = (nown + P - 1) // P
    last_cap = nown - (nblk - 1) * P
    caps = np.full((NCORES, nblk), P, np.int64)
    caps[:, nblk - 1] = last_cap
    deg = np.bincount(dst, minlength=n_nodes)
    order = np.argsort(-deg, kind="stable")
    bins = [[] for _ in range(NCORES * nblk)]
    flat_caps = caps.reshape(-1)
    active = list(range(NCORES * nblk))
    i = 0
    fwd = True
    while i < n_nodes:
        seq = active if fwd else active[::-1]
        for b in seq:
            if i >= n_nodes:
                break
            bins[b].append(order[i])
            i += 1
        fwd = not fwd
        active = [b for b in active if len(bins[b]) < flat_caps[b]]
    perm = np.empty(n_nodes, np.int64)
    k = 0
    for c in range(NCORES):
        for b in range(nblk):
            members = bins[c * nblk + b]
            perm[k:k + len(members)] = members
            k += len(members)
    return perm


def kernel(**inputs):
    x = np.asarray(inputs["x"], np.float32)
    edge_index = np.asarray(inputs["edge_index"])
    _dst0 = np.asarray(edge_index[1]).astype(np.int64)
    perm = _balance_perm(x.shape[0], _dst0)
    inv = np.empty_like(perm)
    inv[perm] = np.arange(perm.size)
    x = x[perm]
    edge_index = inv[np.asarray(edge_index).astype(np.int64)]
    n_nodes, in_f = x.shape
    hid = inputs["Wl2"].shape[0]
    out_f = inputs["Wl4"].shape[1]
    nown = n_nodes // NCORES
    nhB = nown - HALF

    src = np.asarray(edge_index[0]).astype(np.int64)
    dst = np.asarray(edge_index[1]).astype(np.int64)
    deg = np.bincount(dst, minlength=n_nodes).astype(np.float32)
    deginv = (1.0 / np.maximum(deg, 1.0)).astype(np.float32)

    plans = _finalize([Plan(n_nodes, src, dst, c) for c in range(NCORES)])
    pl = plans[0]
    print(f"[kernel] self chunks {pl.stot} remote chunks {pl.rtot}", flush=True)

    import time as _time
    _t0 = _time.perf_counter()
    nc = build_program(n_nodes, in_f, hid, out_f, pl)
    print(f"[kernel] program built in {_time.perf_counter() - _t0:.1f}s", flush=True)
    _t0 = _time.perf_counter()
    nc.compile()
    print(f"[kernel] bacc compile in {_time.perf_counter() - _t0:.1f}s", flush=True)

    xr = np.zeros((n_nodes, 128), np.float16)
    xr[:, :in_f] = x.astype(np.float16)
    xA = np.concatenate([xr[c * nown:c * nown + HALF] for c in range(NCORES)])
    xB = np.concatenate([xr[c * nown + HALF:(c + 1) * nown] for c in range(NCORES)])
    nblk = pl.nblk
    pad_n = nblk * P

    in_maps = []
    for c, p in enumerate(plans):
        xTc = np.zeros((in_f, pad_n), np.float16)
        xTc[:, :nown] = x[c * nown:(c + 1) * nown].T.astype(np.float16)
        dg = np.zeros(pad_n, np.float32)
        dg[:nown] = deginv[c * nown:(c + 1) * nown]
        im = {
            "xT": xTc, "xself": xr[c * nown:(c + 1) * nown],
            "xA": xA, "xB": xB,
            "idxS": p.idxS if p.idxS.size else np.zeros((P, 8), np.int16),
            "dstS": p.dstS,
            "idxR": p.idxR if p.idxR.size else np.zeros((P, 8), np.int16),
            "dstR": p.dstR,
            "deginv": dg,
            "bl4": np.asarray(inputs["bl4"], np.float32),
        }
        for l in (1, 2, 3, 4):
            im[f"Wl{l}"] = np.asarray(inputs[f"Wl{l}"], np.float16)
            im[f"Wr{l}"] = np.asarray(inputs[f"Wr{l}"], np.float16)
        for l in (1, 2, 3):
            im[f"g{l}"] = np.asarray(inputs[f"g{l}"], np.float32)
            im[f"b{l}"] = np.asarray(inputs[f"b{l}"], np.float32)
        in_maps.append(im)

    global LAST_BUILD
    LAST_BUILD = (nc, in_maps)
    results = _execute(nc, in_maps)
    out = np.concatenate([results[c]["out"] for c in range(NCORES)], axis=0)
    out_full = np.empty_like(out)
    out_full[perm] = out
    return out_full.astype(np.float32)


# revision 30
# speedup vs baseline: 1.3026x; 1.2275x over previous
"""DeepGraphSAGE (4x SAGEConv + BN/ReLU) on 8 Trainium2 NeuronCores.

Sharding: nodes partitioned across 8 cores (6250 dst nodes each). Per layer:
  - mean-aggregate neighbor features via dma_gather + one-hot selection
    matmuls accumulating in PSUM (S built on-chip via iota/is_equal).
  - edges 3-way split by source: SELF (own rows, gathered from local h_own
    during the collective window, pre-aggregated into DRAM partials),
    REMOTE-A (first HALF local rows of each owner) and REMOTE-B (rest).
    The hidden-state exchange is two sub-AllGathers (A then B) so remote-A
    gathers start as soon as the first half arrives; all index spaces fit
    int16 without a base split.
  - dense transforms in transposed layout (features on partitions)
  - BatchNorm stats via accumulated sums + tiny cross-core AllReduce
  - next layer's root term precomputed on PE during the collectives.
Hidden tables travel fp8e3 (E3M4); weights/root f16; accumulation fp32.
"""
import sys
import numpy as np

for p in ("/opt/trn_rl_repo",):
    if p not in sys.path:
        sys.path.append(p)

import concourse.bass as bass
import concourse.bacc as bacc
import concourse.mybir as mybir
from concourse.tile import TileContext
from concourse.masks import make_identity

f32 = mybir.dt.float32
f16 = mybir.dt.float16
fp8 = mybir.dt.float8e3
i16 = mybir.dt.int16

NCORES = 8
P = 128
HALF_BLK = 24
HALF = HALF_BLK * P            # 3072 local rows in the A half
EPS = 1e-5
LAST_BUILD = None
TAB_DT = {1: fp8, 2: fp8}


# ---------------------------------------------------------------- host prep
class Plan:
    """Per-core gather plan: self / remote-A / remote-B chunk streams."""

    def __init__(self, n_nodes, src, dst, core):
        nown = n_nodes // NCORES
        self.n_own = nown
        self.nblk = (nown + P - 1) // P
        self.npair = (self.nblk + 1) // 2
        nhB = nown - HALF
        lo = core * nown
        m = (dst >= lo) & (dst < lo + nown)
        es = src[m].astype(np.int64)
        ed = (dst[m] - lo).astype(np.int64)
        order = np.argsort(ed, kind="stable")
        es, ed = es[order], ed[order]
        bounds = np.searchsorted(ed, np.arange(0, self.nblk + 1) * P)
        owner = es // nown
        lr = es % nown
        grp = np.where(owner == core, 0, np.where(lr < HALF, 1, 2))
        gidx = np.where(grp == 0, lr,
                        np.where(grp == 1, owner * HALF + lr,
                                 owner * nhB + (lr - HALF)))
        # sitems[pr][bi] / ritems[pr][gi][bi] = (idx_flat[k*P], dst[P,k])
        self.sitems, self.ritems = [], []
        for pr in range(self.npair):
            blocks = [b for b in (2 * pr, 2 * pr + 1) if b < self.nblk]
            self.sitems.append(
                [self._chunk(es, ed, bounds, b, grp, gidx, 0) for b in blocks])
            self.ritems.append(
                [[self._chunk(es, ed, bounds, b, grp, gidx, gid)
                  for b in blocks] for gid in (1, 2)])

    @staticmethod
    def _chunk(es, ed, bounds, b, grp, gidx, gid):
        e0, e1 = bounds[b], bounds[b + 1]
        msel = grp[e0:e1] == gid
        gs = gidx[e0:e1][msel]
        gd = ed[e0:e1][msel] - b * P
        k = (len(gs) + P - 1) // P
        iv = np.zeros(k * P, np.int16)
        iv[:len(gs)] = gs.astype(np.int16)
        dv = np.full((P, k), -1, np.int16)
        if k:
            dpad = np.full(k * P, -1, np.int64)
            dpad[:len(gd)] = gd
            dv = dpad.reshape(k, P).T.astype(np.int16).copy()
        return iv, dv


def _pack(idx_vals, dst_vals):
    iv = np.concatenate(idx_vals) if idx_vals else np.zeros(0, np.int16)
    w = iv.reshape(-1, 16).T if iv.size else np.zeros((16, 0), np.int16)
    idx16 = np.tile(w, (8, 1)).copy()
    dst16 = (np.concatenate(dst_vals, axis=1).copy()
             if dst_vals else np.zeros((P, 1), np.int16))
    return idx16, dst16


def _finalize(plans):
    """Cross-core pad chunk counts; build packed idx/dst tables + call lists."""
    npair = plans[0].npair

    def padded(item, km):
        iv, dv = item
        k = dv.shape[1] if dv.size else 0
        iv2 = np.zeros(km * P, np.int16)
        iv2[:k * P] = iv
        dv2 = np.full((P, km), -1, np.int16)
        if k:
            dv2[:, :k] = dv
        return iv2, dv2

    kmax_s = [[max(len(p.sitems[pr][bi][0]) // P for p in plans)
               for bi in range(len(plans[0].sitems[pr]))]
              for pr in range(npair)]
    kmax_r = [[[max(len(p.ritems[pr][gi][bi][0]) // P for p in plans)
                for bi in range(len(plans[0].ritems[pr][gi]))]
               for gi in range(2)] for pr in range(npair)]
    for p in plans:
        sidx, sdst, ridx, rdst = [], [], [], []
        for pr in range(npair):
            for bi, km in enumerate(kmax_s[pr]):
                iv, dv = padded(p.sitems[pr][bi], km)
                if km:
                    sidx.append(iv)
                    sdst.append(dv)
            for gi in range(2):
                for bi, km in enumerate(kmax_r[pr][gi]):
                    iv, dv = padded(p.ritems[pr][gi][bi], km)
                    if km:
                        ridx.append(iv)
                        rdst.append(dv)
        p.idxS, p.dstS = _pack(sidx, sdst)
        p.idxR, p.dstR = _pack(ridx, rdst)
    pl = plans[0]
    pl.scalls = [list(kmax_s[pr]) for pr in range(npair)]
    pl.rcalls = [[(gi + 1, list(kmax_r[pr][gi])) for gi in range(2)]
                 for pr in range(npair)]
    pl.schoff, pl.rchoff = [], []
    so = ro = 0
    for pr in range(npair):
        pl.schoff.append(so)
        pl.rchoff.append(ro)
        so += sum(kmax_s[pr])
        ro += sum(sum(ks) for _, ks in pl.rcalls[pr])
    pl.stot, pl.rtot = so, ro
    return plans


# ---------------------------------------------------------------- program
def build_program(n_nodes, in_f, hid, out_f, pl):
    nown = pl.n_own
    nblk = pl.nblk
    pad_n = nblk * P
    ntile = (nown + 511) // 512
    nfc = hid // P
    npair = pl.npair
    nhB = nown - HALF
    dims = [(in_f, hid), (hid, hid), (hid, hid), (hid, out_f)]

    nc = bacc.Bacc("TRN2", target_bir_lowering=False, debug=False,
                   num_devices=NCORES, num_swdge_queues=4)

    # ---- I/O ----
    xT = nc.dram_tensor("xT", [in_f, pad_n], f16, kind="ExternalInput")
    xself_d = nc.dram_tensor("xself", [nown, 128], f16, kind="ExternalInput")
    xA_d = nc.dram_tensor("xA", [NCORES * HALF, 128], f16, kind="ExternalInput")
    xB_d = nc.dram_tensor("xB", [NCORES * nhB, 128], f16, kind="ExternalInput")
    idxS_d = nc.dram_tensor("idxS", [P, max(pl.stot * 8, 8)], i16, kind="ExternalInput")
    dstS_d = nc.dram_tensor("dstS", [P, max(pl.stot, 1)], i16, kind="ExternalInput")
    idxR_d = nc.dram_tensor("idxR", [P, max(pl.rtot * 8, 8)], i16, kind="ExternalInput")
    dstR_d = nc.dram_tensor("dstR", [P, max(pl.rtot, 1)], i16, kind="ExternalInput")
    deginv_d = nc.dram_tensor("deginv", [pad_n], f32, kind="ExternalInput")
    wl_d, wr_d, g_d, b_d = {}, {}, {}, {}
    for l, (fi, fo) in enumerate(dims, start=1):
        wl_d[l] = nc.dram_tensor(f"Wl{l}", [fi, fo], f16, kind="ExternalInput")
        wr_d[l] = nc.dram_tensor(f"Wr{l}", [fi, fo], f16, kind="ExternalInput")
    for l in (1, 2, 3):
        g_d[l] = nc.dram_tensor(f"g{l}", [hid], f32, kind="ExternalInput")
        b_d[l] = nc.dram_tensor(f"b{l}", [hid], f32, kind="ExternalInput")
    bl4_d = nc.dram_tensor("bl4", [out_f], f32, kind="ExternalInput")
    out_d = nc.dram_tensor("out", [nown, out_f], f32, kind="ExternalOutput")

    # ---- internal DRAM ----
    h_own = {l: nc.dram_tensor(f"h{l}_own", [nown, hid], TAB_DT[l]) for l in (1, 2)}
    h_gA = {l: nc.dram_tensor(f"h{l}_gA", [NCORES * HALF, hid], TAB_DT[l],
                              addr_space="Shared") for l in (1, 2)}
    h_gB = {l: nc.dram_tensor(f"h{l}_gB", [NCORES * nhB, hid], TAB_DT[l],
                              addr_space="Shared") for l in (1, 2)}
    y_own = nc.dram_tensor("y_own", [nown, 128], f16)
    y_gA = nc.dram_tensor("y_gA", [NCORES * HALF, 128], f16, addr_space="Shared")
    y_gB = nc.dram_tensor("y_gB", [NCORES * nhB, 128], f16, addr_space="Shared")
    st_in = {l: nc.dram_tensor(f"st{l}_in", [P, 8], f32) for l in (1, 2, 3)}
    st_out = {l: nc.dram_tensor(f"st{l}_out", [P, 8], f32, addr_space="Shared")
              for l in (1, 2, 3)}
    rg = [list(range(NCORES))]

    with TileContext(nc) as tc:
        with (
            tc.tile_pool(name="const", bufs=1) as cp,
            tc.tile_pool(name="sbuf", bufs=2) as sb,
            tc.tile_pool(name="small", bufs=3) as sm,
            tc.tile_pool(name="spool", bufs=3) as sp,
            tc.tile_pool(name="gpool", bufs=3) as gp,
            tc.tile_pool(name="sspool", bufs=2) as ssp,
            tc.tile_pool(name="sgpool", bufs=2) as sgp,
            tc.tile_pool(name="rows", bufs=2) as rp,
            tc.tile_pool(name="psA", bufs=2, space="PSUM") as psA,
            tc.tile_pool(name="psB", bufs=2, space="PSUM") as psB,
            tc.tile_pool(name="psC", bufs=2, space="PSUM") as psC,
        ):
            ident = cp.tile([P, P], f16)
            make_identity(nc, ident[:])
            ident32 = cp.tile([P, P], f32)
            make_identity(nc, ident32[:])
            iota_t = cp.tile([P, P], i16)
            nc.gpsimd.iota(iota_t[:], pattern=[[1, P]], base=0,
                           channel_multiplier=0,
                           allow_small_or_imprecise_dtypes=True)
            deginv_t = cp.tile([P, nblk], f32)
            nc.sync.dma_start(out=deginv_t[:],
                              in_=deginv_d[:].rearrange("(b p) -> p b", p=P))
            idxSc = cp.tile([P, max(pl.stot * 8, 8)], i16)
            nc.sync.dma_start(out=idxSc[:], in_=idxS_d[:, :])
            dstSc = cp.tile([P, max(pl.stot, 1)], i16)
            nc.sync.dma_start(out=dstSc[:], in_=dstS_d[:, :])
            idxRc = cp.tile([P, max(pl.rtot * 8, 8)], i16)
            nc.sync.dma_start(out=idxRc[:], in_=idxR_d[:, :])
            dstRc = cp.tile([P, max(pl.rtot, 1)], i16)
            nc.sync.dma_start(out=dstRc[:], in_=dstR_d[:, :])
            W = {}
            for l, (fi, fo) in enumerate(dims, start=1):
                kc = (fi + P - 1) // P
                for (nm, dram) in (("l", wl_d[l]), ("r", wr_d[l])):
                    for q in range(kc):
                        r0, r1 = q * P, min((q + 1) * P, fi)
                        t = cp.tile([r1 - r0, fo], f16, tag=f"W{nm}{l}_{q}")
                        nc.sync.dma_start(out=t[:], in_=dram[r0:r1, :])
                        W[(nm, l, q)] = t
            gb = {}
            for l in (1, 2, 3):
                for nm, dram in (("g", g_d[l]), ("b", b_d[l])):
                    t = cp.tile([P, nfc], f32, tag=f"{nm}{l}")
                    nc.sync.dma_start(out=t[:], in_=dram[:].rearrange("(c p) -> p c", p=P))
                    gb[(nm, l)] = t
            bl4_t = cp.tile([P, 1], f32)
            nc.sync.dma_start(out=bl4_t[:out_f, :], in_=bl4_d[:, None])

            hT = [cp.tile([P, pad_n], f16, tag=f"hT{q}", name=f"hT{q}") for q in range(nfc)]
            preBN = [cp.tile([P, pad_n], f16, tag=f"preBN{q}", name=f"preBN{q}") for q in range(nfc)]
            if pad_n > nown:
                for q in range(nfc):
                    nc.vector.memset(hT[q][:, nown:pad_n], 0.0)

            qload = [0, 0, 0, 0]  # per-queue gathered-row balance

            def next_queue(rows):
                q = min(range(4), key=lambda i: qload[i])
                qload[q] += rows
                return q

            # ---------------- aggregation ----------------
            def aggregate_pair(pr, tabA, tabB, width, tagsfx, row_elems, dt,
                               selftab=None):
                """Mean-aggregate both blocks of pair pr from remote tables
                plus inline self chunks. Returns f16 tiles per block."""
                groups = pl.rcalls[pr]
                blocks = [b for b in (2 * pr, 2 * pr + 1) if b < nblk]
                ch0 = pl.rchoff[pr]
                rktot = sum(sum(ks) for _, ks in groups)
                out_tiles = []
                stile = g = None
                if rktot:
                    stile = sp.tile([P, rktot, P], dt, tag="S")
                    nc.vector.tensor_tensor(
                        out=stile[:],
                        in0=dstRc[:, ch0:ch0 + rktot].unsqueeze(2)
                            .broadcast_to([P, rktot, P]),
                        in1=iota_t[:].unsqueeze(1).broadcast_to([P, rktot, P]),
                        op=mybir.AluOpType.is_equal)
                    g = gp.tile([P, rktot, row_elems], dt, tag="G")
                    koff = 0
                    for gid, ks in groups:
                        k = sum(ks)
                        if k == 0:
                            continue
                        src_ap = tabA if gid == 1 else tabB
                        parts = ([k] if k <= 4 else [(k + 1) // 2, k // 2])
                        po = 0
                        for kp in parts:
                            if kp == 0:
                                continue
                            o = koff + po
                            nc.gpsimd.dma_gather(
                                out_ap=g[:, o:o + kp, :], in_ap=src_ap,
                                idxs_ap=idxRc[:, (ch0 + o) * 8:(ch0 + o + kp) * 8],
                                num_idxs=kp * P, num_idxs_reg=kp * P,
                                elem_size=row_elems, single_packet=False,
                                queue_num=next_queue(kp * P))
                            po += kp
                        koff += k
                # inline self chunks (layer 1)
                sS = gS = None
                sks = pl.scalls[pr]
                sch0 = pl.schoff[pr]
                if selftab is not None and sum(sks):
                    sktot = sum(sks)
                    sS = ssp.tile([P, sktot, P], dt, tag="Ss")
                    nc.vector.tensor_tensor(
                        out=sS[:],
                        in0=dstSc[:, sch0:sch0 + sktot].unsqueeze(2)
                            .broadcast_to([P, sktot, P]),
                        in1=iota_t[:].unsqueeze(1).broadcast_to([P, sktot, P]),
                        op=mybir.AluOpType.is_equal)
                    gS = sgp.tile([P, sktot, row_elems], dt, tag="Gs")
                    nc.gpsimd.dma_gather(
                        out_ap=gS[:], in_ap=selftab,
                        idxs_ap=idxSc[:, sch0 * 8:(sch0 + sktot) * 8],
                        num_idxs=sktot * P, num_idxs_reg=sktot * P,
                        elem_size=row_elems, single_packet=False,
                        queue_num=next_queue(sktot * P))
                for bi, b in enumerate(blocks):
                    # remote chunk js of this block
                    mm_idx = []
                    koff = 0
                    for gid, ks in groups:
                        pre = 0
                        for i2, k2 in enumerate(ks):
                            if i2 == bi:
                                mm_idx += list(range(koff + pre, koff + pre + k2))
                            pre += k2
                        koff += sum(ks)
                    smm = []
                    if sS is not None:
                        pre = 0
                        for i2, k2 in enumerate(sks):
                            if i2 == bi:
                                smm = list(range(pre, pre + k2))
                            pre += k2
                    nmm = len(mm_idx) + len(smm)
                    if nmm == 0:
                        z = sm.tile([P, width], f16, tag=f"agg{tagsfx}{bi}",
                                    name=f"aggz{bi}")
                        nc.vector.memset(z[:], 0.0)
                        out_tiles.append(z)
                        continue
                    agg_ps = psA.tile([P, 512], f32, tag=f"agg_ps{bi}",
                                      name=f"agg_ps{bi}")
                    mm = 0
                    for j in smm:
                        nc.tensor.matmul(out=agg_ps[:, :width],
                                         lhsT=sS[:, j, :], rhs=gS[:, j, :width],
                                         start=(mm == 0), stop=(mm == nmm - 1))
                        mm += 1
                    for j in mm_idx:
                        nc.tensor.matmul(out=agg_ps[:, :width],
                                         lhsT=stile[:, j, :], rhs=g[:, j, :width],
                                         start=(mm == 0), stop=(mm == nmm - 1))
                        mm += 1
                    asb = sm.tile([P, width], f16, tag=f"agg{tagsfx}{bi}",
                                  name=f"asb{bi}")
                    if b % 2 == 0:
                        nc.scalar.activation(
                            out=asb[:], in_=agg_ps[:, :width],
                            func=mybir.ActivationFunctionType.Copy,
                            scale=deginv_t[:, b:b + 1])
                    else:
                        nc.vector.tensor_scalar(
                            out=asb[:], in0=agg_ps[:, :width],
                            scalar1=deginv_t[:, b:b + 1], scalar2=None,
                            op0=mybir.AluOpType.mult)
                    out_tiles.append(asb)
                return out_tiles

            # ---------------- one SAGE layer with BN+ReLU ----------------
            def layer_123(l, tabA, tabB, fi_chunks, rhs_for_fi, width,
                          row_elems, dt, root_pre=False, selftab=None):
                sums = [sb.tile([P, ntile], f32, tag=f"sums{q}", name=f"sums{q}")
                        for q in range(nfc)]
                sumsqs = [sb.tile([P, ntile], f32, tag=f"sumsq{q}", name=f"sumsq{q}")
                          for q in range(nfc)]
                for nt in range(ntile):
                    ns, ne = nt * 512, min((nt + 1) * 512, nown)
                    nn = ne - ns
                    aggT = (sb.tile([in_f, 512], f16, tag="aggT", name="aggT")
                            if width == in_f else None)
                    aggTq = ([sb.tile([P, 512], f16, tag=f"aggT{q}", name=f"aggT{q}")
                              for q in range(fi_chunks)] if width > in_f else None)
                    pair_tiles = []
                    for pr in (2 * nt, 2 * nt + 1):
                        if pr * 2 < nblk:
                            pair_tiles += aggregate_pair(
                                pr, tabA, tabB, width, "sb", row_elems, dt,
                                selftab=selftab)
                    for bi, b in enumerate(range(nt * 4, min(nt * 4 + 4, nblk))):
                        asb = pair_tiles[bi]
                        tp = psB.tile([P, 512], f16, tag="tp")
                        if width == in_f:
                            nc.tensor.matmul(out=tp[:width, bi * P:(bi + 1) * P],
                                             lhsT=asb[:], rhs=ident[:],
                                             is_transpose=True)
                            nc.vector.tensor_copy(out=aggT[:width, bi * P:(bi + 1) * P],
                                                  in_=tp[:width, bi * P:(bi + 1) * P])
                        else:
                            for q in range(fi_chunks):
                                nc.tensor.matmul(out=tp[:, q * P:(q + 1) * P],
                                                 lhsT=asb[:, q * P:(q + 1) * P],
                                                 rhs=ident[:], is_transpose=True)
                                if (bi + q) % 2 == 0:
                                    nc.scalar.activation(
                                        out=aggTq[q][:, bi * P:(bi + 1) * P],
                                        in_=tp[:, q * P:(q + 1) * P],
                                        func=mybir.ActivationFunctionType.Copy)
                                else:
                                    nc.vector.tensor_copy(
                                        out=aggTq[q][:, bi * P:(bi + 1) * P],
                                        in_=tp[:, q * P:(q + 1) * P])
                    for fo in range(nfc):
                        dps = psC.tile([P, 512], f32, tag="dense")
                        nmm = fi_chunks if root_pre else 2 * fi_chunks
                        mm = 0
                        for q in range(fi_chunks):
                            rhs_agg = (aggT[:width, :nn] if width == in_f
                                       else aggTq[q][:, :nn])
                            nc.tensor.matmul(out=dps[:, :nn],
                                             lhsT=W[("l", l, q)][:, fo * P:(fo + 1) * P],
                                             rhs=rhs_agg, start=(mm == 0),
                                             stop=(mm == nmm - 1))
                            mm += 1
                            if not root_pre:
                                nc.tensor.matmul(out=dps[:, :nn],
                                                 lhsT=W[("r", l, q)][:, fo * P:(fo + 1) * P],
                                                 rhs=rhs_for_fi(q, ns, ne),
                                                 start=False, stop=(mm == nmm - 1))
                                mm += 1
                        if root_pre:
                            nc.vector.scalar_tensor_tensor(
                                out=preBN[fo][:, ns:ne], in0=dps[:, :nn],
                                scalar=1.0, in1=preBN[fo][:, ns:ne],
                                op0=mybir.AluOpType.mult,
                                op1=mybir.AluOpType.add,
                                accum_out=sums[fo][:, nt:nt + 1])
                            scr = sm.tile([P, 512], f16, tag="scr", name="scr")
                            nc.scalar.activation(
                                out=scr[:, :nn], in_=preBN[fo][:, ns:ne],
                                func=mybir.ActivationFunctionType.Square,
                                accum_out=sumsqs[fo][:, nt:nt + 1])
                        else:
                            scr = sm.tile([P, 512], f16, tag="scr", name="scr")
                            nc.scalar.activation(
                                out=scr[:, :nn], in_=dps[:, :nn],
                                func=mybir.ActivationFunctionType.Square,
                                accum_out=sumsqs[fo][:, nt:nt + 1])
                            nc.vector.tensor_scalar(
                                out=preBN[fo][:, ns:ne], in0=dps[:, :nn],
                                scalar1=1.0, scalar2=None,
                                op0=mybir.AluOpType.mult,
                                op1=mybir.AluOpType.add,
                                accum_out=sums[fo][:, nt:nt + 1])
                # ---- BN statistics + cross-core allreduce ----
                pack = sb.tile([P, 8], f32, tag="pack")
                for q in range(nfc):
                    nc.vector.reduce_sum(out=pack[:, 2 * q:2 * q + 1],
                                         in_=sums[q][:], axis=mybir.AxisListType.X)
                    nc.vector.reduce_sum(out=pack[:, 2 * q + 1:2 * q + 2],
                                         in_=sumsqs[q][:], axis=mybir.AxisListType.X)
                nc.sync.dma_start(out=st_in[l][:, :], in_=pack[:])
                nc.gpsimd.collective_compute(
                    "AllReduce", mybir.AluOpType.add, replica_groups=rg,
                    ins=[st_in[l][:, :]], outs=[st_out[l][:, :]],
                )
                red = sb.tile([P, 8], f32, tag="red")
                nc.sync.dma_start(out=red[:], in_=st_out[l][:, :])
                scale = sb.tile([P, nfc], f32, tag="scale")
                shift = sb.tile([P, nfc], f32, tag="shift")
                inv_n = 1.0 / float(n_nodes)
                for q in range(nfc):
                    mu = sb.tile([P, 1], f32, tag="mu")
                    var = sb.tile([P, 1], f32, tag="var")
                    nc.vector.tensor_scalar(out=mu[:], in0=red[:, 2 * q:2 * q + 1],
                                            scalar1=inv_n, scalar2=None,
                                            op0=mybir.AluOpType.mult)
                    nc.vector.tensor_scalar(out=var[:], in0=red[:, 2 * q + 1:2 * q + 2],
                                            scalar1=inv_n, scalar2=None,
                                            op0=mybir.AluOpType.mult)
                    musq = sb.tile([P, 1], f32, tag="musq")
                    nc.vector.tensor_tensor(out=musq[:], in0=mu[:], in1=mu[:],
                                            op=mybir.AluOpType.mult)
                    nc.vector.tensor_tensor(out=var[:], in0=var[:], in1=musq[:],
                                            op=mybir.AluOpType.subtract)
                    nc.vector.tensor_scalar(out=var[:], in0=var[:], scalar1=EPS,
                                            scalar2=None, op0=mybir.AluOpType.add)
                    nc.vector.reciprocal(out=var[:], in_=var[:])
                    rs = sb.tile([P, 1], f32, tag="rs")
                    nc.scalar.activation(out=rs[:], in_=var[:],
                                         func=mybir.ActivationFunctionType.Sqrt)
                    nc.vector.tensor_tensor(out=scale[:, q:q + 1], in0=rs[:],
                                            in1=gb[("g", l)][:, q:q + 1],
                                            op=mybir.AluOpType.mult)
                    nc.vector.tensor_tensor(out=musq[:], in0=mu[:],
                                            in1=scale[:, q:q + 1],
                                            op=mybir.AluOpType.mult)
                    nc.vector.tensor_tensor(out=shift[:, q:q + 1],
                                            in0=gb[("b", l)][:, q:q + 1], in1=musq[:],
                                            op=mybir.AluOpType.subtract)
                # ---- BN apply + ReLU -> hT ----
                for q in range(nfc):
                    nc.scalar.activation(
                        out=hT[q][:, 0:nown], in_=preBN[q][:, 0:nown],
                        func=mybir.ActivationFunctionType.Relu,
                        bias=shift[:, q:q + 1], scale=scale[:, q:q + 1],
                    )
                if l == 3:
                    return
                # ---- rows + sub-AllGathers (A half first) ----
                def rows_range(b_lo, b_hi):
                    for b2 in range(b_lo, b_hi, 2):
                        bl2 = [b for b in (b2, b2 + 1) if b < b_hi]
                        w2 = len(bl2) * hid
                        tpr = psB.tile([P, 1024], f16, tag="tp")
                        for bi, b in enumerate(bl2):
                            for q in range(nfc):
                                nc.tensor.matmul(
                                    out=tpr[:, bi * hid + q * P:bi * hid + (q + 1) * P],
                                    lhsT=hT[q][:, b * P:(b + 1) * P],
                                    rhs=ident[:], is_transpose=True)
                        rows = rp.tile([P, 1024], TAB_DT[l], tag="rows")
                        if (b2 // 2) % 2 == 0:
                            nc.vector.tensor_copy(out=rows[:, :w2], in_=tpr[:, :w2])
                        else:
                            nc.scalar.activation(
                                out=rows[:, :w2], in_=tpr[:, :w2],
                                func=mybir.ActivationFunctionType.Copy)
                        for bi, b in enumerate(bl2):
                            ns, ne = b * P, min((b + 1) * P, nown)
                            nc.sync.dma_start(
                                out=h_own[l][ns:ne, :],
                                in_=rows[:ne - ns, bi * hid:bi * hid + hid])
                rows_range(0, HALF_BLK)
                nc.gpsimd.collective_compute(
                    "AllGather", mybir.AluOpType.bypass, replica_groups=rg,
                    ins=[h_own[l][0:HALF, :]], outs=[h_gA[l][:, :]],
                )
                rows_range(HALF_BLK, nblk)
                nc.gpsimd.collective_compute(
                    "AllGather", mybir.AluOpType.bypass, replica_groups=rg,
                    ins=[h_own[l][HALF:nown, :]], outs=[h_gB[l][:, :]],
                )
                # next layer's root term (PE overlaps the AllGathers)
                nl = l + 1
                for nt2 in range(ntile):
                    ns2, ne2 = nt2 * 512, min((nt2 + 1) * 512, nown)
                    nn2 = ne2 - ns2
                    for fo in range(nfc):
                        rps = psC.tile([P, 512], f32, tag="dense")
                        for q in range(nfc):
                            nc.tensor.matmul(
                                out=rps[:, :nn2],
                                lhsT=W[("r", nl, q)][:, fo * P:(fo + 1) * P],
                                rhs=hT[q][:, ns2:ne2],
                                start=(q == 0), stop=(q == nfc - 1))
                        if (nt2 + fo) % 2 == 0:
                            nc.vector.tensor_copy(out=preBN[fo][:, ns2:ne2],
                                                  in_=rps[:, :nn2])
                        else:
                            nc.scalar.activation(
                                out=preBN[fo][:, ns2:ne2], in_=rps[:, :nn2],
                                func=mybir.ActivationFunctionType.Copy)

            # ================= layer 1 =================
            def xT_rhs(q, ns, ne):
                xt = sm.tile([in_f, 512], f16, tag="xTt", name="xTt")
                nc.sync.dma_start(out=xt[:, :ne - ns], in_=xT[:, ns:ne])
                return xt[:, :ne - ns]
            layer_123(1, xA_d[:, :], xB_d[:, :], 1, xT_rhs, in_f, 128, f16,
                      selftab=xself_d[:, :])
            # ================= layers 2,3 =================
            for l in (2, 3):
                layer_123(l, h_gA[l - 1][:, :], h_gB[l - 1][:, :], nfc,
                          None, hid, hid, TAB_DT[l - 1], root_pre=True,
                          selftab=h_own[l - 1][:, :])
            # ================= layer 4 =================
            for nt in range(ntile):
                ns, ne = nt * 512, min((nt + 1) * 512, nown)
                nn = ne - ns
                yps = psC.tile([P, 512], f32, tag="dense")
                for q in range(nfc):
                    nc.tensor.matmul(out=yps[:out_f, :nn],
                                     lhsT=W[("l", 4, q)][:, :out_f],
                                     rhs=hT[q][:, ns:ne],
                                     start=(q == 0), stop=(q == nfc - 1))
                ysb = sb.tile([P, 512], f16, tag="ysb")
                if nt % 2 == 0:
                    nc.scalar.activation(out=ysb[:out_f, :nn], in_=yps[:out_f, :nn],
                                         func=mybir.ActivationFunctionType.Copy)
                else:
                    nc.vector.tensor_copy(out=ysb[:out_f, :nn], in_=yps[:out_f, :nn])
                for bi in range((nn + P - 1) // P):
                    b0 = bi * P
                    b1 = min(b0 + P, nn)
                    tpy = psB.tile([P, 512], f16, tag="tp")
                    nc.tensor.matmul(out=tpy[:b1 - b0, :out_f],
                                     lhsT=ysb[:out_f, b0:b1],
                                     rhs=ident[:out_f, :out_f],
                                     is_transpose=True)
                    yr = sb.tile([P, 128], f16, tag="yrows")
                    # pad cols out_f..128 are never read by the agg matmuls
                    if bi % 2 == 0:
                        nc.vector.tensor_copy(out=yr[:b1 - b0, :out_f],
                                              in_=tpy[:b1 - b0, :out_f])
                    else:
                        nc.scalar.activation(out=yr[:b1 - b0, :out_f],
                                             in_=tpy[:b1 - b0, :out_f],
                                             func=mybir.ActivationFunctionType.Copy)
                    nc.sync.dma_start(out=y_own[ns + b0:ns + b1, :out_f],
                                      in_=yr[:b1 - b0, :out_f])
            nc.gpsimd.collective_compute(
                "AllGather", mybir.AluOpType.bypass, replica_groups=rg,
                ins=[y_own[0:HALF, :]], outs=[y_gA[:, :]],
            )
            nc.gpsimd.collective_compute(
                "AllGather", mybir.AluOpType.bypass, replica_groups=rg,
                ins=[y_own[HALF:nown, :]], outs=[y_gB[:, :]],
            )
            # root term h3 @ Wr4 -> preBN[0]
            for nt in range(ntile):
                ns, ne = nt * 512, min((nt + 1) * 512, nown)
                nn = ne - ns
                rps4 = psC.tile([P, 512], f32, tag="dense")
                for q in range(nfc):
                    nc.tensor.matmul(out=rps4[:out_f, :nn],
                                     lhsT=W[("r", 4, q)][:, :out_f],
                                     rhs=hT[q][:, ns:ne],
                                     start=(q == 0), stop=(q == nfc - 1))
                if nt % 2 == 0:
                    nc.vector.tensor_copy(out=preBN[0][:out_f, ns:ne],
                                          in_=rps4[:out_f, :nn])
                else:
                    nc.scalar.activation(out=preBN[0][:out_f, ns:ne],
                                         in_=rps4[:out_f, :nn],
                                         func=mybir.ActivationFunctionType.Copy)
            # final: out = mean-agg(y) + h3 @ Wr4 + bl4
            for nt in range(ntile):
                ns, ne = nt * 512, min((nt + 1) * 512, nown)
                nn = ne - ns
                agg4T = sb.tile([P, 512], f16, tag="agg4T")
                pair_tiles4 = []
                for pr in (2 * nt, 2 * nt + 1):
                    if pr * 2 < nblk:
                        pair_tiles4 += aggregate_pair(pr, y_gA[:, :],
                                                      y_gB[:, :],
                                                      out_f, "4", 128, f16,
                                                      selftab=y_own[:, :])
                for bi, b in enumerate(range(nt * 4, min(nt * 4 + 4, nblk))):
                    asb = pair_tiles4[bi]
                    tp = psB.tile([P, 512], f16, tag="tp")
                    nc.tensor.matmul(out=tp[:out_f, bi * P:(bi + 1) * P],
                                     lhsT=asb[:], rhs=ident[:], is_transpose=True)
                    if bi % 2 == 0:
                        nc.scalar.activation(out=agg4T[:out_f, bi * P:(bi + 1) * P],
                                             in_=tp[:out_f, bi * P:(bi + 1) * P],
                                             func=mybir.ActivationFunctionType.Copy)
                    else:
                        nc.vector.tensor_copy(out=agg4T[:out_f, bi * P:(bi + 1) * P],
                                              in_=tp[:out_f, bi * P:(bi + 1) * P])
                osb = sb.tile([P, 512], f32, tag="osb")
                nc.vector.scalar_tensor_tensor(
                    out=osb[:out_f, :nn], in0=preBN[0][:out_f, ns:ne],
                    scalar=bl4_t[:out_f, 0:1], in1=agg4T[:out_f, :nn],
                    op0=mybir.AluOpType.add, op1=mybir.AluOpType.add)
                for bi in range((nn + P - 1) // P):
                    b0, b1 = bi * P, min(bi * P + P, nn)
                    tpo = psB.tile([P, 512], f32, tag="tp")
                    nc.tensor.matmul(out=tpo[:b1 - b0, :out_f],
                                     lhsT=osb[:out_f, b0:b1],
                                     rhs=ident32[:out_f, :out_f],
                                     is_transpose=True)
                    orow = sb.tile([P, out_f], f32, tag="orow")
                    if bi % 2 == 0:
                        nc.vector.tensor_copy(out=orow[:b1 - b0, :],
                                              in_=tpo[:b1 - b0, :out_f])
                    else:
                        nc.scalar.activation(out=orow[:b1 - b0, :],
                                             in_=tpo[:b1 - b0, :out_f],
                                             func=mybir.ActivationFunctionType.Copy)
                    nc.sync.dma_start(out=out_d[ns + b0:ns + b1, :],
                                      in_=orow[:b1 - b0, :])
    return nc


def _execute(nc, in_maps):
    from concourse.bass_utils import run_bass_kernel_spmd
    res = run_bass_kernel_spmd(nc, in_maps, list(range(NCORES)))
    return [res.results[c] for c in range(NCORES)]


def _balance_perm(n_nodes, dst):
    """Permutation new->old assigning nodes to (core, block) bins so the
    per-block-position degree sums align across cores (kills most of the
    cross-core kmax padding in the gather plans). Snake-deal by degree."""
    nown = n_nodes // NCORES
    nblk = (nown + P - 1) // P
    last_cap = nown - (nblk - 1) * P
    caps = np.full((NCORES, nblk), P, np.int64)
    caps[:, nblk - 1] = last_cap
    deg = np.bincount(dst, minlength=n_nodes)
    order = np.argsort(-deg, kind="stable")
    bins = [[] for _ in range(NCORES * nblk)]
    flat_caps = caps.reshape(-1)
    active = list(range(NCORES * nblk))
    i = 0
    fwd = True
    while i < n_nodes:
        seq = active if fwd else active[::-1]
        for b in seq:
            if i >= n_nodes:
                break
            bins[b].append(order[i])
            i += 1
        fwd = not fwd
        active = [b for b in active if len(bins[b]) < flat_caps[b]]
    perm = np.empty(n_nodes, np.int64)
    k = 0
    for c in range(NCORES):
        for b in range(nblk):
            members = bins[c * nblk + b]
            perm[k:k + len(members)] = members
            k += len(members)
    return perm


def kernel(**inputs):
    x = np.asarray(inputs["x"], np.float32)
    edge_index = np.asarray(inputs["edge_index"])
    _dst0 = np.asarray(edge_index[1]).astype(np.int64)
    perm = _balance_perm(x.shape[0], _dst0)
    inv = np.empty_like(perm)
    inv[perm] = np.arange(perm.size)
    x = x[perm]
    edge_index = inv[np.asarray(edge_index).astype(np.int64)]
    n_nodes, in_f = x.shape
    hid = inputs["Wl2"].shape[0]
    out_f = inputs["Wl4"].shape[1]
    nown = n_nodes // NCORES
    nhB = nown - HALF

    src = np.asarray(edge_index[0]).astype(np.int64)
    dst = np.asarray(edge_index[1]).astype(np.int64)
    deg = np.bincount(dst, minlength=n_nodes).astype(np.float32)
    deginv = (1.0 / np.maximum(deg, 1.0)).astype(np.float32)

    plans = _finalize([Plan(n_nodes, src, dst, c) for c in range(NCORES)])
    pl = plans[0]
    print(f"[kernel] self chunks {pl.stot} remote chunks {pl.rtot}", flush=True)

    import time as _time
    _t0 = _time.perf_counter()
    nc = build_program(n_nodes, in_f, hid, out_f, pl)
    print(f"[kernel] program built in {_time.perf_counter() - _t0:.1f}s", flush=True)
    _t0 = _time.perf_counter()
    nc.compile()
    print(f"[kernel] bacc compile in {_time.perf_counter() - _t0:.1f}s", flush=True)

    xr = np.zeros((n_nodes, 128), np.float16)
    xr[:, :in_f] = x.astype(np.float16)
    xA = np.concatenate([xr[c * nown:c * nown + HALF] for c in range(NCORES)])
    xB = np.concatenate([xr[c * nown + HALF:(c + 1) * nown] for c in range(NCORES)])
    nblk = pl.nblk
    pad_n = nblk * P

    in_maps = []
    for c, p in enumerate(plans):
        xTc = np.zeros((in_f, pad_n), np.float16)
        xTc[:, :nown] = x[c * nown:(c + 1) * nown].T.astype(np.float16)
        dg = np.zeros(pad_n, np.float32)
        dg[:nown] = deginv[c * nown:(c + 1) * nown]
        im = {
            "xT": xTc, "xself": xr[c * nown:(c + 1) * nown],
            "xA": xA, "xB": xB,
            "idxS": p.idxS if p.idxS.size else np.zeros((P, 8), np.int16),
            "dstS": p.dstS,
            "idxR": p.idxR if p.idxR.size else np.zeros((P, 8), np.int16),
            "dstR": p.dstR,
            "deginv": dg,
            "bl4": np.asarray(inputs["bl4"], np.float32),
        }
        for l in (1, 2, 3, 4):
            im[f"Wl{l}"] = np.asarray(inputs[f"Wl{l}"], np.float16)
            im[f"Wr{l}"] = np.asarray(inputs[f"Wr{l}"], np.float16)
        for l in (1, 2, 3):
            im[f"g{l}"] = np.asarray(inputs[f"g{l}"], np.float32)
            im[f"b{l}"] = np.asarray(inputs[f"b{l}"], np.float32)
        in_maps.append(im)

    global LAST_BUILD
    LAST_BUILD = (nc, in_maps)
    results = _execute(nc, in_maps)
    out = np.concatenate([results[c]["out"] for c in range(NCORES)], axis=0)
    out_full = np.empty_like(out)
    out_full[perm] = out
    return out_full.astype(np.float32)
